# revision 33
# baseline (speedup 1.0000x reference)
"""AttnDecoderRNN single-step kernel for 8 Trainium2 NeuronCores.

Parallelization (tensor-parallel over the vocab/hidden dims):
  - emb table row-sharded 8 ways; device-side masked indirect-DMA gather +
    AllReduce broadcasts the embedded row to all cores.
  - attention + combine replicated (W_attn, W_comb, encoder_outputs on every
    core) so each core computes the full attn/x vectors with no collective.
  - GRU gate weights (w_ih/w_hh) output-row-sharded: each core computes the
    gates and h_new only for its 128-dim slice of H, locally.
  - W_out column(H)-sharded: each core computes partial logits for the FULL
    (padded) vocab from its h_new slice; one AllReduce sums the partials,
    then every core runs the log_softmax epilogue identically.

All weights are host pre-packed into the chunk-partition layout [128, ...]
the matmuls consume (pure layout prep; all FLOPs run on device).
"""

import os
import sys

sys.path.insert(0, "/opt/trn_rl_repo")

import numpy as np
import ml_dtypes

import concourse.bass as bass
import concourse.bacc as bacc
import concourse.mybir as mybir
import concourse.tile as tile
from concourse.bass_utils import run_bass_kernel_spmd
from concourse.masks import make_identity

F32 = mybir.dt.float32
F32R = mybir.dt.float32r
F16 = mybir.dt.float16
I32 = mybir.dt.int32
ALU = mybir.AluOpType
ACT = mybir.ActivationFunctionType
AX = mybir.AxisListType

H = 1024          # hidden size
L = 40            # encoder slots
V = 50257         # vocab
M = 8             # cores
VP = 53248        # padded vocab = 416 * 128
VE = VP // M      # emb rows per core      (6656)
NB = VP // 128    # vocab blocks           (416)
NC_H = H // 128   # h chunks               (8)
TW = 4096         # W_out tile width (cols per streamed tile)
NT = VP // TW     # streamed W_out tiles   (26)
PAD_BIAS = -30000.0

# ---------------------------------------------------------------- device ---


def build_program():
    nc = bacc.Bacc("TRN2", target_bir_lowering=False, num_devices=M)

    tok_t = nc.dram_tensor("tok", [2, 1], I32, kind="ExternalInput")
    base_t = nc.dram_tensor("base", [2, 1], I32, kind="ExternalInput")
    emb_t = nc.dram_tensor("emb_shard", [VE, H], F32, kind="ExternalInput")
    hnat_t = nc.dram_tensor("h_nat", [H], F32, kind="ExternalInput")
    hsl_t = nc.dram_tensor("h_slice_row", [1, 128], F32, kind="ExternalInput")
    enc_t = nc.dram_tensor("enc", [L, H], F32, kind="ExternalInput")
    wa_t = nc.dram_tensor("wa_cp", [128, 16 * L], F32, kind="ExternalInput")
    ba_t = nc.dram_tensor("b_attn", [1, L], F32, kind="ExternalInput")
    wc_t = nc.dram_tensor("wc_cp", [128, 16 * H], F32R, kind="ExternalInput")
    bc_t = nc.dram_tensor("b_comb_row", [1, H], F32, kind="ExternalInput")
    wih_t = nc.dram_tensor("wih_cp", [128, NC_H * 384], F32R, kind="ExternalInput")
    whh_t = nc.dram_tensor("whh_cp", [128, NC_H * 384], F32R, kind="ExternalInput")
    bih_t = nc.dram_tensor("bih_row", [1, 384], F32, kind="ExternalInput")
    bhh_t = nc.dram_tensor("bhh_row", [1, 384], F32, kind="ExternalInput")
    wo_t = nc.dram_tensor("wo_cp", [128, VP], F16, kind="ExternalInput")
    bo_t = nc.dram_tensor("bo_cp", [128, NB], F32, kind="ExternalInput")

    logp_t = nc.dram_tensor("out_logp", [VP], F32, kind="ExternalOutput")
    outh_t = nc.dram_tensor("out_h", [128], F32, kind="ExternalOutput")
    outa_t = nc.dram_tensor("out_attn", [L], F32, kind="ExternalOutput")

    with tile.TileContext(nc) as tc:
        with (
            tc.tile_pool(name="dram", bufs=1, space="DRAM") as dram,
            tc.tile_pool(name="w", bufs=1) as wpool,
            tc.tile_pool(name="s", bufs=1) as spool,
            tc.tile_pool(name="wo", bufs=8) as wo_pool,
        ):
            # ---- token -> masked embedding row gather, AllReduce first ----
            tok_sb = spool.tile([2, 1], I32)
            base_sb = spool.tile([2, 1], I32)
            nc.sync.dma_start(tok_sb[:], tok_t[:])
            nc.sync.dma_start(base_sb[:], base_t[:])
            d_sb = spool.tile([2, 1], I32)
            nc.vector.tensor_tensor(
                out=d_sb[:], in0=tok_sb[:], in1=base_sb[:], op=ALU.subtract
            )
            dcl = spool.tile([2, 1], I32)
            nc.vector.tensor_scalar(
                out=dcl[:], in0=d_sb[:], scalar1=0, scalar2=VE - 1,
                op0=ALU.max, op1=ALU.min,
            )
            d_f = spool.tile([2, 1], F32)
            nc.vector.tensor_copy(out=d_f[:], in_=d_sb[:])
            m1 = spool.tile([2, 1], F32)
            m2 = spool.tile([2, 1], F32)
            msk = spool.tile([2, 1], F32)
            nc.vector.tensor_scalar(
                out=m1[:], in0=d_f[:], scalar1=0.0, scalar2=None, op0=ALU.is_ge
            )
            nc.vector.tensor_scalar(
                out=m2[:], in0=d_f[:], scalar1=float(VE - 1), scalar2=None,
                op0=ALU.is_le,
            )
            nc.vector.tensor_tensor(out=msk[:], in0=m1[:], in1=m2[:], op=ALU.mult)
            gath = spool.tile([2, H], F32)
            nc.gpsimd.indirect_dma_start(
                out=gath[:],
                out_offset=None,
                in_=emb_t[:, :],
                in_offset=bass.IndirectOffsetOnAxis(ap=dcl[:, :1], axis=0),
            )
            erow = spool.tile([1, H], F32)
            nc.vector.tensor_scalar(
                out=erow[:], in0=gath[0:1, :], scalar1=msk[0:1, 0:1],
                scalar2=None, op0=ALU.mult,
            )
            em_in = dram.tile([1, H], F32)
            em_out = dram.tile([1, H], F32)
            nc.sync.dma_start(em_in[:], erow[:])
            nc.gpsimd.collective_compute(
                "AllReduce", ALU.add, replica_groups=[list(range(M))],
                ins=[em_in.opt()], outs=[em_out.opt()],
            )

            # ---- small weights into SBUF -------------------------------
            wa_sb = wpool.tile([128, 16 * L], F32)
            ba_sb = wpool.tile([1, L], F32)
            enc_sb = wpool.tile([L, H], F32)
            wc_sb = wpool.tile([128, 16 * H], F32R)
            bc_sb = wpool.tile([1, H], F32)
            wih_sb = wpool.tile([128, NC_H * 384], F32R)
            whh_sb = wpool.tile([128, NC_H * 384], F32R)
            bih_sb = wpool.tile([1, 384], F32)
            bhh_sb = wpool.tile([1, 384], F32)
            hsl_sb = wpool.tile([1, 128], F32)
            bo_sb = wpool.tile([128, NB], F32)
            ident = wpool.tile([128, 128], F32)
            ones1 = wpool.tile([1, 128], F32)
            nc.gpsimd.memset(ones1[:], 1.0)
            nc.sync.dma_start(wa_sb[:], wa_t[:])
            nc.sync.dma_start(ba_sb[:], ba_t[:])
            nc.sync.dma_start(enc_sb[:], enc_t[:])
            nc.scalar.dma_start(wc_sb[:], wc_t[:])
            nc.scalar.dma_start(bc_sb[:], bc_t[:])
            nc.sync.dma_start(wih_sb[:], wih_t[:])
            nc.scalar.dma_start(whh_sb[:], whh_t[:])
            nc.sync.dma_start(bih_sb[:], bih_t[:])
            nc.scalar.dma_start(bhh_sb[:], bhh_t[:])
            nc.sync.dma_start(hsl_sb[:], hsl_t[:])
            nc.sync.dma_start(bo_sb[:], bo_t[:])
            make_identity(nc, ident[:])

            hin = spool.tile([128, NC_H], F32)
            nc.sync.dma_start(hin[:], hnat_t[:].rearrange("(c p) -> p c", p=128))
            ein = spool.tile([128, NC_H], F32)
            nc.sync.dma_start(
                ein[:], em_out[:].rearrange("a (c p) -> p (a c)", p=128)
            )
            ein_r = spool.tile([128, NC_H], F32R)
            nc.vector.tensor_copy(out=ein_r[:], in_=ein[:])
            hin_r = spool.tile([128, NC_H], F32R)
            nc.vector.tensor_copy(out=hin_r[:], in_=hin[:])

            # ---- attention --------------------------------------------
            with tc.tile_pool(name="ps_a", bufs=1, space="PSUM") as ps_a:
                sc_ps = ps_a.tile([1, L], F32, space="PSUM")
                for c in range(16):
                    lhsT = ein[:, c : c + 1] if c < 8 else hin[:, c - 8 : c - 7]
                    nc.tensor.matmul(
                        sc_ps[0:1, 0:L], lhsT=lhsT,
                        rhs=wa_sb[:, c * L : (c + 1) * L],
                        start=(c == 0), stop=(c == 15),
                    )
                sc_sb = spool.tile([1, L], F32)
                nc.vector.tensor_tensor(
                    out=sc_sb[:], in0=sc_ps[:], in1=ba_sb[:], op=ALU.add
                )
                mx = spool.tile([1, 1], F32)
                nc.vector.tensor_reduce(
                    out=mx[:], in_=sc_sb[:], axis=AX.X, op=ALU.max
                )
                nmx = spool.tile([1, 1], F32)
                nc.vector.tensor_scalar(
                    out=nmx[:], in0=mx[:], scalar1=-1.0, scalar2=None, op0=ALU.mult
                )
                aw_e = spool.tile([1, L], F32)
                ssum = spool.tile([1, 1], F32)
                nc.scalar.activation(
                    out=aw_e[:], in_=sc_sb[:], func=ACT.Exp,
                    bias=nmx[0:1, 0:1], accum_out=ssum[0:1, 0:1],
                )
                rs = spool.tile([1, 1], F32)
                nc.vector.reciprocal(out=rs[:], in_=ssum[:])
                aw = spool.tile([1, L], F32)
                nc.vector.tensor_scalar(
                    out=aw[:], in0=aw_e[:], scalar1=rs[0:1, 0:1], scalar2=None,
                    op0=ALU.mult,
                )
                nc.sync.dma_start(
                    outa_t[:].rearrange("(a l) -> a l", a=1), aw[0:1, 0:L]
                )
                awT_ps = ps_a.tile([L, 1], F32, space="PSUM")
                nc.tensor.transpose(
                    out=awT_ps[0:L, 0:1], in_=aw[0:1, 0:L], identity=ident[0:1, 0:1]
                )
                awT = spool.tile([L, 1], F32)
                nc.vector.tensor_copy(out=awT[:], in_=awT_ps[:])
                aa_ps = ps_a.tile([128, NC_H], F32, space="PSUM")
                for c2 in range(NC_H):
                    nc.tensor.matmul(
                        aa_ps[:, c2 : c2 + 1],
                        lhsT=enc_sb[:, c2 * 128 : (c2 + 1) * 128],
                        rhs=awT[0:L, 0:1], start=True, stop=True,
                    )
                aa_sb = spool.tile([128, NC_H], F32R)
                nc.vector.tensor_copy(out=aa_sb[:], in_=aa_ps[:])

            # ---- combine + GRU: vector-stationary fp32r matmuls -------
            # x.T [1, H] = sum_kc cin_kc.T @ WcT slab (weights moving, N=512)
            with tc.tile_pool(name="ps_g", bufs=1, space="PSUM") as ps_g:
                xt_ps0 = ps_g.tile([1, 512], F32, space="PSUM")
                xt_ps1 = ps_g.tile([1, 512], F32, space="PSUM")
                for kc in range(16):
                    lhsT = (
                        ein_r[:, kc : kc + 1]
                        if kc < 8
                        else aa_sb[:, kc - 8 : kc - 7]
                    )
                    for half, xps in ((0, xt_ps0), (1, xt_ps1)):
                        nc.tensor.matmul(
                            xps[0:1, :],
                            lhsT=lhsT,
                            rhs=wc_sb[
                                :, kc * H + half * 512 : kc * H + half * 512 + 512
                            ],
                            start=(kc == 0), stop=(kc == 15),
                        )
                xt = spool.tile([1, H], F32)
                nc.vector.tensor_tensor(
                    out=xt[:, 0:512], in0=xt_ps0[0:1, :], in1=bc_sb[:, 0:512],
                    op=ALU.add,
                )
                nc.vector.tensor_tensor(
                    out=xt[:, 512:H], in0=xt_ps1[0:1, :], in1=bc_sb[:, 512:H],
                    op=ALU.add,
                )
                xr = spool.tile([1, H], F32)
                nc.scalar.activation(out=xr[:], in_=xt[:], func=ACT.Relu)
                # transpose x back to chunk-partition [128, 8] for lhsT use
                xc_ps = ps_g.tile([128, NC_H], F32, space="PSUM")
                for c in range(NC_H):
                    nc.tensor.transpose(
                        out=xc_ps[:, c : c + 1],
                        in_=xr[0:1, c * 128 : (c + 1) * 128],
                        identity=ident[0:1, 0:1],
                    )
                x_cp = spool.tile([128, NC_H], F32R)
                nc.vector.tensor_copy(out=x_cp[:], in_=xc_ps[:])

                gi_ps = ps_g.tile([1, 384], F32, space="PSUM")
                gh_ps = ps_g.tile([1, 384], F32, space="PSUM")
                for kc in range(NC_H):
                    nc.tensor.matmul(
                        gi_ps[0:1, :],
                        lhsT=x_cp[:, kc : kc + 1],
                        rhs=wih_sb[:, kc * 384 : (kc + 1) * 384],
                        start=(kc == 0), stop=(kc == NC_H - 1),
                    )
                for kc in range(NC_H):
                    nc.tensor.matmul(
                        gh_ps[0:1, :],
                        lhsT=hin_r[:, kc : kc + 1],
                        rhs=whh_sb[:, kc * 384 : (kc + 1) * 384],
                        start=(kc == 0), stop=(kc == NC_H - 1),
                    )
                gi = spool.tile([1, 384], F32)
                gh = spool.tile([1, 384], F32)
                nc.vector.tensor_tensor(
                    out=gi[:], in0=gi_ps[:], in1=bih_sb[:], op=ALU.add
                )
                nc.vector.tensor_tensor(
                    out=gh[:], in0=gh_ps[:], in1=bhh_sb[:], op=ALU.add
                )
            rz_in = spool.tile([1, 256], F32)
            nc.vector.tensor_tensor(
                out=rz_in[:], in0=gi[:, 0:256], in1=gh[:, 0:256], op=ALU.add
            )
            rz = spool.tile([1, 256], F32)
            nc.scalar.activation(out=rz[:], in_=rz_in[:], func=ACT.Sigmoid)
            rhn = spool.tile([1, 128], F32)
            nc.vector.tensor_tensor(
                out=rhn[:], in0=rz[:, 0:128], in1=gh[:, 256:384], op=ALU.mult
            )
            nin = spool.tile([1, 128], F32)
            nc.vector.tensor_tensor(
                out=nin[:], in0=gi[:, 256:384], in1=rhn[:], op=ALU.add
            )
            nn = spool.tile([1, 128], F32)
            nc.scalar.activation(out=nn[:], in_=nin[:], func=ACT.Tanh)
            hmn = spool.tile([1, 128], F32)
            nc.vector.tensor_tensor(
                out=hmn[:], in0=hsl_sb[:], in1=nn[:], op=ALU.subtract
            )
            zh = spool.tile([1, 128], F32)
            nc.vector.tensor_tensor(
                out=zh[:], in0=rz[:, 128:256], in1=hmn[:], op=ALU.mult
            )
            hnew = spool.tile([1, 128], F32)
            nc.vector.tensor_tensor(out=hnew[:], in0=nn[:], in1=zh[:], op=ALU.add)
            nc.sync.dma_start(outh_t[:].rearrange("(a p) -> a p", a=1), hnew[:])

            # ---- W_out partial logits: h stationary, W moving fp32r ---
            ar_in = dram.tile([VP], F32)
            ar_out = dram.tile([VP], F32)
            with tc.tile_pool(name="ps_h", bufs=1, space="PSUM") as ps_h:
                hc_ps = ps_h.tile([128, 1], F32, space="PSUM")
                nc.tensor.transpose(
                    out=hc_ps[:, 0:1], in_=hnew[0:1, :], identity=ident[0:1, 0:1]
                )
                hnew_cp = spool.tile([128, 1], F16)
                nc.vector.tensor_copy(out=hnew_cp[:], in_=hc_ps[:])
            with (
                tc.tile_pool(name="ps_lg", bufs=8, space="PSUM") as ps_lg,
                tc.tile_pool(name="lgstage", bufs=3) as lgstage,
            ):
                for t in range(NT):
                    wtile = wo_pool.tile([128, TW], F16)
                    # alternate the weight stream across both HWDGE engines
                    weng = nc.sync if t % 2 == 0 else nc.scalar
                    oeng = nc.scalar if t % 2 == 0 else nc.sync
                    weng.dma_start(wtile[:], wo_t[:, t * TW : (t + 1) * TW])
                    for s in range(TW // 512):
                        lgt_ps = ps_lg.tile(
                            [1, 512], F32, space="PSUM", tag="lgt"
                        )
                        nc.tensor.matmul(
                            lgt_ps[0:1, :],
                            lhsT=hnew_cp[:, 0:1],
                            rhs=wtile[:, s * 512 : (s + 1) * 512],
                            start=True, stop=True,
                        )
                        lgt_sb = lgstage.tile([1, 512], F32, tag="lgs")
                        if s % 2 == 0:
                            nc.vector.tensor_copy(out=lgt_sb[:], in_=lgt_ps[:])
                        else:
                            nc.scalar.copy(out=lgt_sb[:], in_=lgt_ps[:])
                        off = t * TW + s * 512
                        nc.gpsimd.dma_start(
                            ar_in[off : off + 512].rearrange(
                                "(a n) -> a n", a=1
                            ),
                            lgt_sb[:],
                        )
            nc.gpsimd.collective_compute(
                "AllReduce", ALU.add, replica_groups=[list(range(M))],
                ins=[ar_in.opt()], outs=[ar_out.opt()],
            )

            # ---- log_softmax epilogue (identical on all cores) --------
            lgf = spool.tile([128, NB], F32)
            nc.sync.dma_start(
                lgf[:], ar_out[:].rearrange("(p b) -> p b", p=128)
            )
            lgb = spool.tile([128, NB], F32)
            nc.vector.tensor_tensor(out=lgb[:], in0=lgf[:], in1=bo_sb[:], op=ALU.add)
            rmx = spool.tile([128, 1], F32)
            nc.vector.tensor_reduce(out=rmx[:], in_=lgb[:], axis=AX.X, op=ALU.max)
            gmx = spool.tile([1, 1], F32)
            ngmx = spool.tile([128, 1], F32)
            ex = spool.tile([128, NB], F32)
            rsum = spool.tile([128, 1], F32)
            gsum = spool.tile([1, 1], F32)
            lz = spool.tile([1, 1], F32)
            logz = spool.tile([1, 1], F32)
            logz_sb = spool.tile([128, 1], F32)
            logp = spool.tile([128, NB], F32)
            with tc.tile_pool(name="ps_b", bufs=1, space="PSUM") as ps_b:
                rmxT_ps = ps_b.tile([1, 128], F32, space="PSUM")
                nc.tensor.transpose(
                    out=rmxT_ps[0:1, :], in_=rmx[:, 0:1], identity=ident[:, :]
                )
                nc.vector.tensor_reduce(
                    out=gmx[:], in_=rmxT_ps[0:1, :], axis=AX.X, op=ALU.max
                )
                gmxb_ps = ps_b.tile([128, 1], F32, space="PSUM")
                nc.tensor.matmul(
                    gmxb_ps[:, 0:1], lhsT=ones1[0:1, :], rhs=gmx[0:1, 0:1],
                    start=True, stop=True,
                )
                nc.vector.tensor_scalar(
                    out=ngmx[:], in0=gmxb_ps[:, 0:1],
                    scalar1=-1.0, scalar2=None, op0=ALU.mult,
                )
                nc.scalar.activation(
                    out=ex[:], in_=lgb[:], func=ACT.Exp,
                    bias=ngmx[:, 0:1], accum_out=rsum[:, 0:1],
                )
                rsumT_ps = ps_b.tile([1, 128], F32, space="PSUM")
                nc.tensor.transpose(
                    out=rsumT_ps[0:1, :], in_=rsum[:, 0:1], identity=ident[:, :]
                )
                nc.vector.tensor_reduce(
                    out=gsum[:], in_=rsumT_ps[0:1, :], axis=AX.X, op=ALU.add
                )
                nc.scalar.activation(out=lz[:], in_=gsum[:], func=ACT.Ln)
                nc.vector.tensor_tensor(
                    out=logz[:], in0=gmx[:], in1=lz[:], op=ALU.add
                )
                logzb_ps = ps_b.tile([128, 1], F32, space="PSUM")
                nc.tensor.matmul(
                    logzb_ps[:, 0:1], lhsT=ones1[0:1, :], rhs=logz[0:1, 0:1],
                    start=True, stop=True,
                )
                nc.vector.tensor_copy(out=logz_sb[:], in_=logzb_ps[:, 0:1])
            nc.vector.tensor_scalar(
                out=logp[:], in0=lgb[:],
                scalar1=logz_sb[:, 0:1],
                scalar2=None, op0=ALU.subtract,
            )
            nc.sync.dma_start(
                logp_t[:].rearrange("(p b) -> p b", p=128), logp[:]
            )

    nc.compile()
    return nc


# ------------------------------------------------------------------ host ---

_NC = None


def _get_nc():
    global _NC
    if _NC is None:
        _NC = build_program()
    return _NC


def prep_in_maps(input_tok, hidden, encoder_outputs, emb, W_attn, b_attn,
                 W_comb, b_comb, w_ih, w_hh, b_ih, b_hh, W_out, b_out):
    f = lambda a: np.ascontiguousarray(np.asarray(a, dtype=np.float32))
    emb = f(emb)
    W_attn, b_attn = f(W_attn), f(b_attn)
    W_comb, b_comb = f(W_comb), f(b_comb)
    w_ih, w_hh, b_ih, b_hh = f(w_ih), f(w_hh), f(b_ih), f(b_hh)
    W_out, b_out = f(W_out), f(b_out)
    hidden = f(hidden)
    enc = f(encoder_outputs)
    tok = int(np.asarray(input_tok).reshape(-1)[0])

    # pre-packed shared (replicated) weights
    wa_cp = np.ascontiguousarray(
        W_attn.T.reshape(16, 128, L).transpose(1, 0, 2).reshape(128, 16 * L)
    )
    wc_cp = np.ascontiguousarray(
        W_comb.T.reshape(16, 128, H).transpose(1, 0, 2).reshape(128, 16 * H)
    )
    b_comb_row = np.ascontiguousarray(b_comb.reshape(1, H))
    b_attn_r = b_attn.reshape(1, L)

    W_out_pad = np.zeros((VP, H), np.float32)
    W_out_pad[:V] = W_out
    WoT = np.ascontiguousarray(W_out_pad.T.astype(np.float16))  # [H, VP]
    b_out_pad = np.full(VP, PAD_BIAS, np.float32)
    b_out_pad[:V] = b_out
    bo_cp = np.ascontiguousarray(b_out_pad.reshape(128, NB))  # row p = vocab p*NB..

    h_nat = hidden.reshape(H)
    wihT = np.ascontiguousarray(w_ih.T)              # [H, 3H]
    whhT = np.ascontiguousarray(w_hh.T)

    emb_pad_last = np.zeros((VE, H), np.float32)
    emb_pad_last[: V - 7 * VE] = emb[7 * VE :]

    in_maps = []
    for c in range(M):
        s = slice(c * 128, (c + 1) * 128)
        wih_c = np.concatenate(
            [wihT[:, g * H + c * 128 : g * H + (c + 1) * 128] for g in range(3)],
            axis=1,
        )  # [H, 384]
        whh_c = np.concatenate(
            [whhT[:, g * H + c * 128 : g * H + (c + 1) * 128] for g in range(3)],
            axis=1,
        )
        wih_cp = np.ascontiguousarray(
            wih_c.reshape(NC_H, 128, 384).transpose(1, 0, 2).reshape(128, NC_H * 384)
        )
        whh_cp = np.ascontiguousarray(
            whh_c.reshape(NC_H, 128, 384).transpose(1, 0, 2).reshape(128, NC_H * 384)
        )
        bih_row = np.concatenate(
            [b_ih[g * H + c * 128 : g * H + (c + 1) * 128] for g in range(3)]
        ).reshape(1, 384)
        bhh_row = np.concatenate(
            [b_hh[g * H + c * 128 : g * H + (c + 1) * 128] for g in range(3)]
        ).reshape(1, 384)
        emb_shard = emb[c * VE : (c + 1) * VE] if c < 7 else emb_pad_last
        in_maps.append({
            "tok": np.full((2, 1), tok, np.int32),
            "base": np.full((2, 1), c * VE, np.int32),
            "emb_shard": np.ascontiguousarray(emb_shard),
            "h_nat": h_nat,
            "h_slice_row": np.ascontiguousarray(h_nat[s].reshape(1, 128)),
            "enc": enc,
            "wa_cp": wa_cp,
            "b_attn": b_attn_r,
            "wc_cp": wc_cp,
            "b_comb_row": b_comb_row,
            "wih_cp": wih_cp,
            "whh_cp": whh_cp,
            "bih_row": np.ascontiguousarray(bih_row),
            "bhh_row": np.ascontiguousarray(bhh_row),
            "wo_cp": np.ascontiguousarray(WoT[s]),
            "bo_cp": bo_cp,
        })
    return in_maps


def assemble(results):
    logp = results[0]["out_logp"][:V].reshape(1, V).astype(np.float32)
    h_new = np.concatenate(
        [results[c]["out_h"] for c in range(M)]
    ).reshape(1, 1, H).astype(np.float32)
    attn = results[0]["out_attn"].reshape(1, L).astype(np.float32)
    return logp, h_new, attn


def kernel(**inputs):
    nc = _get_nc()
    in_maps = prep_in_maps(**inputs)
    kw = {}
    if os.environ.get("KERNEL_TRACE"):
        kw = dict(trace=True, tmpdir=os.environ.get("KERNEL_TRACE_DIR") or None)
    res = run_bass_kernel_spmd(nc, in_maps, list(range(M)), **kw)
    if os.environ.get("KERNEL_TRACE"):
        print(f"HW exec time: {res.exec_time_ns} ns")
    return assemble(res.results)


# revision 34
# speedup vs baseline: 1.0431x; 1.0431x over previous
"""AttnDecoderRNN single-step kernel for 8 Trainium2 NeuronCores.

Parallelization (tensor-parallel over the vocab/hidden dims):
  - emb table row-sharded 8 ways; device-side masked indirect-DMA gather +
    AllReduce broadcasts the embedded row to all cores.
  - attention + combine replicated (W_attn, W_comb, encoder_outputs on every
    core) so each core computes the full attn/x vectors with no collective.
  - GRU gate weights (w_ih/w_hh) output-row-sharded: each core computes the
    gates and h_new only for its 128-dim slice of H, locally.
  - W_out column(H)-sharded: each core computes partial logits for the FULL
    (padded) vocab from its h_new slice; one AllReduce sums the partials,
    then every core runs the log_softmax epilogue identically.

All weights are host pre-packed into the chunk-partition layout [128, ...]
the matmuls consume (pure layout prep; all FLOPs run on device).
"""

import os
import sys

sys.path.insert(0, "/opt/trn_rl_repo")

import numpy as np
import ml_dtypes

import concourse.bass as bass
import concourse.bacc as bacc
import concourse.mybir as mybir
import concourse.tile as tile
from concourse.bass_utils import run_bass_kernel_spmd
from concourse.masks import make_identity

F32 = mybir.dt.float32
F32R = mybir.dt.float32r
F16 = mybir.dt.float16
I32 = mybir.dt.int32
ALU = mybir.AluOpType
ACT = mybir.ActivationFunctionType
AX = mybir.AxisListType

H = 1024          # hidden size
L = 40            # encoder slots
V = 50257         # vocab
M = 8             # cores
VP = 53248        # padded vocab = 416 * 128
VE = VP // M      # emb rows per core      (6656)
NB = VP // 128    # vocab blocks           (416)
NC_H = H // 128   # h chunks               (8)
TW = 4096         # W_out tile width (cols per streamed tile)
NT = VP // TW     # streamed W_out tiles   (26)
PAD_BIAS = -30000.0

# ---------------------------------------------------------------- device ---


def build_program():
    nc = bacc.Bacc("TRN2", target_bir_lowering=False, num_devices=M)

    tok_t = nc.dram_tensor("tok", [2, 1], I32, kind="ExternalInput")
    base_t = nc.dram_tensor("base", [2, 1], I32, kind="ExternalInput")
    emb_t = nc.dram_tensor("emb_shard", [VE, H], F32, kind="ExternalInput")
    hnat_t = nc.dram_tensor("h_nat", [H], F32, kind="ExternalInput")
    hsl_t = nc.dram_tensor("h_slice_row", [1, 128], F32, kind="ExternalInput")
    enc_t = nc.dram_tensor("enc", [L, H], F32, kind="ExternalInput")
    wa_t = nc.dram_tensor("wa_cp", [128, 16 * L], F32, kind="ExternalInput")
    ba_t = nc.dram_tensor("b_attn", [1, L], F32, kind="ExternalInput")
    wc_t = nc.dram_tensor("wc_cp", [128, 16 * H], F32R, kind="ExternalInput")
    bc_t = nc.dram_tensor("b_comb_row", [1, H], F32, kind="ExternalInput")
    wih_t = nc.dram_tensor("wih_cp", [128, NC_H * 384], F32R, kind="ExternalInput")
    whh_t = nc.dram_tensor("whh_cp", [128, NC_H * 384], F32R, kind="ExternalInput")
    bih_t = nc.dram_tensor("bih_row", [1, 384], F32, kind="ExternalInput")
    bhh_t = nc.dram_tensor("bhh_row", [1, 384], F32, kind="ExternalInput")
    wo_t = nc.dram_tensor("wo_cp", [128, VP], F16, kind="ExternalInput")
    bo_t = nc.dram_tensor("bo_cp", [128, NB], F32, kind="ExternalInput")

    logp_t = nc.dram_tensor("out_logp", [VP], F32, kind="ExternalOutput")
    outh_t = nc.dram_tensor("out_h", [128], F32, kind="ExternalOutput")
    outa_t = nc.dram_tensor("out_attn", [L], F32, kind="ExternalOutput")

    with tile.TileContext(nc) as tc:
        with (
            tc.tile_pool(name="dram", bufs=1, space="DRAM") as dram,
            tc.tile_pool(name="w", bufs=1) as wpool,
            tc.tile_pool(name="s", bufs=1) as spool,
            tc.tile_pool(name="wo", bufs=7) as wo_pool,
        ):
            # ---- token -> masked embedding row gather, AllReduce first ----
            tok_sb = spool.tile([2, 1], I32)
            base_sb = spool.tile([2, 1], I32)
            nc.sync.dma_start(tok_sb[:], tok_t[:])
            nc.sync.dma_start(base_sb[:], base_t[:])
            d_sb = spool.tile([2, 1], I32)
            nc.vector.tensor_tensor(
                out=d_sb[:], in0=tok_sb[:], in1=base_sb[:], op=ALU.subtract
            )
            dcl = spool.tile([2, 1], I32)
            nc.vector.tensor_scalar(
                out=dcl[:], in0=d_sb[:], scalar1=0, scalar2=VE - 1,
                op0=ALU.max, op1=ALU.min,
            )
            d_f = spool.tile([2, 1], F32)
            nc.vector.tensor_copy(out=d_f[:], in_=d_sb[:])
            m1 = spool.tile([2, 1], F32)
            m2 = spool.tile([2, 1], F32)
            msk = spool.tile([2, 1], F32)
            nc.vector.tensor_scalar(
                out=m1[:], in0=d_f[:], scalar1=0.0, scalar2=None, op0=ALU.is_ge
            )
            nc.vector.tensor_scalar(
                out=m2[:], in0=d_f[:], scalar1=float(VE - 1), scalar2=None,
                op0=ALU.is_le,
            )
            nc.vector.tensor_tensor(out=msk[:], in0=m1[:], in1=m2[:], op=ALU.mult)
            gath = spool.tile([2, H], F32)
            nc.gpsimd.indirect_dma_start(
                out=gath[:],
                out_offset=None,
                in_=emb_t[:, :],
                in_offset=bass.IndirectOffsetOnAxis(ap=dcl[:, :1], axis=0),
            )
            erow = spool.tile([1, H], F32)
            nc.vector.tensor_scalar(
                out=erow[:], in0=gath[0:1, :], scalar1=msk[0:1, 0:1],
                scalar2=None, op0=ALU.mult,
            )
            em_in = dram.tile([1, H], F32)
            em_out = dram.tile([1, H], F32)
            nc.sync.dma_start(em_in[:], erow[:])
            nc.gpsimd.collective_compute(
                "AllReduce", ALU.add, replica_groups=[list(range(M))],
                ins=[em_in.opt()], outs=[em_out.opt()],
            )

            # ---- small weights into SBUF -------------------------------
            wa_sb = wpool.tile([128, 16 * L], F32)
            ba_sb = wpool.tile([1, L], F32)
            enc_sb = wpool.tile([L, H], F32)
            wc_sb = wpool.tile([128, 16 * H], F32R)
            bc_sb = wpool.tile([1, H], F32)
            wih_sb = wpool.tile([128, NC_H * 384], F32R)
            whh_sb = wpool.tile([128, NC_H * 384], F32R)
            bih_sb = wpool.tile([1, 384], F32)
            bhh_sb = wpool.tile([1, 384], F32)
            hsl_sb = wpool.tile([1, 128], F32)
            bo_sb = wpool.tile([128, NB], F32)
            ident = wpool.tile([128, 128], F32)
            ones1 = wpool.tile([1, 128], F32)
            nc.gpsimd.memset(ones1[:], 1.0)
            nc.sync.dma_start(wa_sb[:], wa_t[:])
            nc.sync.dma_start(ba_sb[:], ba_t[:])
            nc.sync.dma_start(enc_sb[:], enc_t[:])
            nc.scalar.dma_start(wc_sb[:], wc_t[:])
            nc.scalar.dma_start(bc_sb[:], bc_t[:])
            nc.sync.dma_start(wih_sb[:], wih_t[:])
            nc.scalar.dma_start(whh_sb[:], whh_t[:])
            nc.sync.dma_start(bih_sb[:], bih_t[:])
            nc.scalar.dma_start(bhh_sb[:], bhh_t[:])
            nc.sync.dma_start(hsl_sb[:], hsl_t[:])
            nc.sync.dma_start(bo_sb[:], bo_t[:])
            make_identity(nc, ident[:])

            hin = spool.tile([128, NC_H], F32)
            nc.sync.dma_start(hin[:], hnat_t[:].rearrange("(c p) -> p c", p=128))
            ein = spool.tile([128, NC_H], F32)
            nc.sync.dma_start(
                ein[:], em_out[:].rearrange("a (c p) -> p (a c)", p=128)
            )
            ein_r = spool.tile([128, NC_H], F32R)
            nc.vector.tensor_copy(out=ein_r[:], in_=ein[:])
            hin_r = spool.tile([128, NC_H], F32R)
            nc.vector.tensor_copy(out=hin_r[:], in_=hin[:])

            # ---- attention --------------------------------------------
            with tc.tile_pool(name="ps_a", bufs=1, space="PSUM") as ps_a:
                sc_ps = ps_a.tile([1, L], F32, space="PSUM")
                for c in range(16):
                    lhsT = ein[:, c : c + 1] if c < 8 else hin[:, c - 8 : c - 7]
                    nc.tensor.matmul(
                        sc_ps[0:1, 0:L], lhsT=lhsT,
                        rhs=wa_sb[:, c * L : (c + 1) * L],
                        start=(c == 0), stop=(c == 15),
                    )
                sc_sb = spool.tile([1, L], F32)
                nc.vector.tensor_tensor(
                    out=sc_sb[:], in0=sc_ps[:], in1=ba_sb[:], op=ALU.add
                )
                mx = spool.tile([1, 1], F32)
                nc.vector.tensor_reduce(
                    out=mx[:], in_=sc_sb[:], axis=AX.X, op=ALU.max
                )
                nmx = spool.tile([1, 1], F32)
                nc.vector.tensor_scalar(
                    out=nmx[:], in0=mx[:], scalar1=-1.0, scalar2=None, op0=ALU.mult
                )
                aw_e = spool.tile([1, L], F32)
                ssum = spool.tile([1, 1], F32)
                nc.scalar.activation(
                    out=aw_e[:], in_=sc_sb[:], func=ACT.Exp,
                    bias=nmx[0:1, 0:1], accum_out=ssum[0:1, 0:1],
                )
                rs = spool.tile([1, 1], F32)
                nc.vector.reciprocal(out=rs[:], in_=ssum[:])
                aw = spool.tile([1, L], F32)
                nc.vector.tensor_scalar(
                    out=aw[:], in0=aw_e[:], scalar1=rs[0:1, 0:1], scalar2=None,
                    op0=ALU.mult,
                )
                nc.sync.dma_start(
                    outa_t[:].rearrange("(a l) -> a l", a=1), aw[0:1, 0:L]
                )
                awT_ps = ps_a.tile([L, 1], F32, space="PSUM")
                nc.tensor.transpose(
                    out=awT_ps[0:L, 0:1], in_=aw[0:1, 0:L], identity=ident[0:1, 0:1]
                )
                awT = spool.tile([L, 1], F32)
                nc.vector.tensor_copy(out=awT[:], in_=awT_ps[:])
                aa_ps = ps_a.tile([128, NC_H], F32, space="PSUM")
                for c2 in range(NC_H):
                    nc.tensor.matmul(
                        aa_ps[:, c2 : c2 + 1],
                        lhsT=enc_sb[:, c2 * 128 : (c2 + 1) * 128],
                        rhs=awT[0:L, 0:1], start=True, stop=True,
                    )
                aa_sb = spool.tile([128, NC_H], F32R)
                nc.vector.tensor_copy(out=aa_sb[:], in_=aa_ps[:])

            # ---- combine + GRU: vector-stationary fp32r matmuls -------
            # x.T [1, H] = sum_kc cin_kc.T @ WcT slab (weights moving, N=512)
            with tc.tile_pool(name="ps_g", bufs=1, space="PSUM") as ps_g:
                xt_ps0 = ps_g.tile([1, 512], F32, space="PSUM")
                xt_ps1 = ps_g.tile([1, 512], F32, space="PSUM")
                for kc in range(16):
                    lhsT = (
                        ein_r[:, kc : kc + 1]
                        if kc < 8
                        else aa_sb[:, kc - 8 : kc - 7]
                    )
                    for half, xps in ((0, xt_ps0), (1, xt_ps1)):
                        nc.tensor.matmul(
                            xps[0:1, :],
                            lhsT=lhsT,
                            rhs=wc_sb[
                                :, kc * H + half * 512 : kc * H + half * 512 + 512
                            ],
                            start=(kc == 0), stop=(kc == 15),
                        )
                xt = spool.tile([1, H], F32)
                nc.vector.tensor_tensor(
                    out=xt[:, 0:512], in0=xt_ps0[0:1, :], in1=bc_sb[:, 0:512],
                    op=ALU.add,
                )
                nc.vector.tensor_tensor(
                    out=xt[:, 512:H], in0=xt_ps1[0:1, :], in1=bc_sb[:, 512:H],
                    op=ALU.add,
                )
                xr = spool.tile([1, H], F32)
                nc.scalar.activation(out=xr[:], in_=xt[:], func=ACT.Relu)
                # transpose x back to chunk-partition [128, 8] for lhsT use
                xc_ps = ps_g.tile([128, NC_H], F32, space="PSUM")
                for c in range(NC_H):
                    nc.tensor.transpose(
                        out=xc_ps[:, c : c + 1],
                        in_=xr[0:1, c * 128 : (c + 1) * 128],
                        identity=ident[0:1, 0:1],
                    )
                x_cp = spool.tile([128, NC_H], F32R)
                nc.vector.tensor_copy(out=x_cp[:], in_=xc_ps[:])

                gi_ps = ps_g.tile([1, 384], F32, space="PSUM")
                gh_ps = ps_g.tile([1, 384], F32, space="PSUM")
                for kc in range(NC_H):
                    nc.tensor.matmul(
                        gi_ps[0:1, :],
                        lhsT=x_cp[:, kc : kc + 1],
                        rhs=wih_sb[:, kc * 384 : (kc + 1) * 384],
                        start=(kc == 0), stop=(kc == NC_H - 1),
                    )
                for kc in range(NC_H):
                    nc.tensor.matmul(
                        gh_ps[0:1, :],
                        lhsT=hin_r[:, kc : kc + 1],
                        rhs=whh_sb[:, kc * 384 : (kc + 1) * 384],
                        start=(kc == 0), stop=(kc == NC_H - 1),
                    )
                gi = spool.tile([1, 384], F32)
                gh = spool.tile([1, 384], F32)
                nc.vector.tensor_tensor(
                    out=gi[:], in0=gi_ps[:], in1=bih_sb[:], op=ALU.add
                )
                nc.vector.tensor_tensor(
                    out=gh[:], in0=gh_ps[:], in1=bhh_sb[:], op=ALU.add
                )
            rz_in = spool.tile([1, 256], F32)
            nc.vector.tensor_tensor(
                out=rz_in[:], in0=gi[:, 0:256], in1=gh[:, 0:256], op=ALU.add
            )
            rz = spool.tile([1, 256], F32)
            nc.scalar.activation(out=rz[:], in_=rz_in[:], func=ACT.Sigmoid)
            rhn = spool.tile([1, 128], F32)
            nc.vector.tensor_tensor(
                out=rhn[:], in0=rz[:, 0:128], in1=gh[:, 256:384], op=ALU.mult
            )
            nin = spool.tile([1, 128], F32)
            nc.vector.tensor_tensor(
                out=nin[:], in0=gi[:, 256:384], in1=rhn[:], op=ALU.add
            )
            nn = spool.tile([1, 128], F32)
            nc.scalar.activation(out=nn[:], in_=nin[:], func=ACT.Tanh)
            hmn = spool.tile([1, 128], F32)
            nc.vector.tensor_tensor(
                out=hmn[:], in0=hsl_sb[:], in1=nn[:], op=ALU.subtract
            )
            zh = spool.tile([1, 128], F32)
            nc.vector.tensor_tensor(
                out=zh[:], in0=rz[:, 128:256], in1=hmn[:], op=ALU.mult
            )
            hnew = spool.tile([1, 128], F32)
            nc.vector.tensor_tensor(out=hnew[:], in0=nn[:], in1=zh[:], op=ALU.add)
            nc.sync.dma_start(outh_t[:].rearrange("(a p) -> a p", a=1), hnew[:])

            # ---- W_out partial logits: h stationary, W moving fp32r ---
            ar_in = dram.tile([VP], F32)
            ar_out = dram.tile([VP], F32)
            with tc.tile_pool(name="ps_h", bufs=1, space="PSUM") as ps_h:
                hc_ps = ps_h.tile([128, 1], F32, space="PSUM")
                nc.tensor.transpose(
                    out=hc_ps[:, 0:1], in_=hnew[0:1, :], identity=ident[0:1, 0:1]
                )
                hnew_cp = spool.tile([128, 1], F16)
                nc.vector.tensor_copy(out=hnew_cp[:], in_=hc_ps[:])
            with (
                tc.tile_pool(name="ps_lg", bufs=8, space="PSUM") as ps_lg,
                tc.tile_pool(name="lgstage", bufs=6) as lgstage,
            ):
                for t in range(NT):
                    wtile = wo_pool.tile([128, TW], F16)
                    # alternate the weight stream across both HWDGE engines
                    weng = nc.sync if t % 2 == 0 else nc.scalar
                    oeng = nc.scalar if t % 2 == 0 else nc.sync
                    weng.dma_start(wtile[:], wo_t[:, t * TW : (t + 1) * TW])
                    for s in range(TW // 512):
                        lgt_ps = ps_lg.tile(
                            [1, 512], F32, space="PSUM", tag="lgt"
                        )
                        nc.tensor.matmul(
                            lgt_ps[0:1, :],
                            lhsT=hnew_cp[:, 0:1],
                            rhs=wtile[:, s * 512 : (s + 1) * 512],
                            start=True, stop=True,
                        )
                        lgt_sb = lgstage.tile([1, 512], F32, tag="lgs")
                        nc.vector.tensor_copy(out=lgt_sb[:], in_=lgt_ps[:])
                        off = t * TW + s * 512
                        nc.gpsimd.dma_start(
                            ar_in[off : off + 512].rearrange(
                                "(a n) -> a n", a=1
                            ),
                            lgt_sb[:],
                        )
            nc.gpsimd.collective_compute(
                "AllReduce", ALU.add, replica_groups=[list(range(M))],
                ins=[ar_in.opt()], outs=[ar_out.opt()],
            )

            # ---- log_softmax epilogue (identical on all cores) --------
            lgf = spool.tile([128, NB], F32)
            nc.sync.dma_start(
                lgf[:], ar_out[:].rearrange("(p b) -> p b", p=128)
            )
            lgb = spool.tile([128, NB], F32)
            nc.vector.tensor_tensor(out=lgb[:], in0=lgf[:], in1=bo_sb[:], op=ALU.add)
            rmx = spool.tile([128, 1], F32)
            nc.vector.tensor_reduce(out=rmx[:], in_=lgb[:], axis=AX.X, op=ALU.max)
            gmx = spool.tile([1, 1], F32)
            ngmx = spool.tile([128, 1], F32)
            ex = spool.tile([128, NB], F32)
            rsum = spool.tile([128, 1], F32)
            gsum = spool.tile([1, 1], F32)
            lz = spool.tile([1, 1], F32)
            logz = spool.tile([1, 1], F32)
            logz_sb = spool.tile([128, 1], F32)
            logp = spool.tile([128, NB], F32)
            with tc.tile_pool(name="ps_b", bufs=1, space="PSUM") as ps_b:
                rmxT_ps = ps_b.tile([1, 128], F32, space="PSUM")
                nc.tensor.transpose(
                    out=rmxT_ps[0:1, :], in_=rmx[:, 0:1], identity=ident[:, :]
                )
                nc.vector.tensor_reduce(
                    out=gmx[:], in_=rmxT_ps[0:1, :], axis=AX.X, op=ALU.max
                )
                gmxb_ps = ps_b.tile([128, 1], F32, space="PSUM")
                nc.tensor.matmul(
                    gmxb_ps[:, 0:1], lhsT=ones1[0:1, :], rhs=gmx[0:1, 0:1],
                    start=True, stop=True,
                )
                nc.vector.tensor_scalar(
                    out=ngmx[:], in0=gmxb_ps[:, 0:1],
                    scalar1=-1.0, scalar2=None, op0=ALU.mult,
                )
                nc.scalar.activation(
                    out=ex[:], in_=lgb[:], func=ACT.Exp,
                    bias=ngmx[:, 0:1], accum_out=rsum[:, 0:1],
                )
                rsumT_ps = ps_b.tile([1, 128], F32, space="PSUM")
                nc.tensor.transpose(
                    out=rsumT_ps[0:1, :], in_=rsum[:, 0:1], identity=ident[:, :]
                )
                nc.vector.tensor_reduce(
                    out=gsum[:], in_=rsumT_ps[0:1, :], axis=AX.X, op=ALU.add
                )
                nc.scalar.activation(out=lz[:], in_=gsum[:], func=ACT.Ln)
                nc.vector.tensor_tensor(
                    out=logz[:], in0=gmx[:], in1=lz[:], op=ALU.add
                )
                logzb_ps = ps_b.tile([128, 1], F32, space="PSUM")
                nc.tensor.matmul(
                    logzb_ps[:, 0:1], lhsT=ones1[0:1, :], rhs=logz[0:1, 0:1],
                    start=True, stop=True,
                )
                nc.vector.tensor_copy(out=logz_sb[:], in_=logzb_ps[:, 0:1])
            nc.vector.tensor_scalar(
                out=logp[:], in0=lgb[:],
                scalar1=logz_sb[:, 0:1],
                scalar2=None, op0=ALU.subtract,
            )
            nc.sync.dma_start(
                logp_t[:].rearrange("(p b) -> p b", p=128), logp[:]
            )

    nc.compile()
    return nc


# ------------------------------------------------------------------ host ---

_NC = None


def _get_nc():
    global _NC
    if _NC is None:
        _NC = build_program()
    return _NC


def prep_in_maps(input_tok, hidden, encoder_outputs, emb, W_attn, b_attn,
                 W_comb, b_comb, w_ih, w_hh, b_ih, b_hh, W_out, b_out):
    f = lambda a: np.ascontiguousarray(np.asarray(a, dtype=np.float32))
    emb = f(emb)
    W_attn, b_attn = f(W_attn), f(b_attn)
    W_comb, b_comb = f(W_comb), f(b_comb)
    w_ih, w_hh, b_ih, b_hh = f(w_ih), f(w_hh), f(b_ih), f(b_hh)
    W_out, b_out = f(W_out), f(b_out)
    hidden = f(hidden)
    enc = f(encoder_outputs)
    tok = int(np.asarray(input_tok).reshape(-1)[0])

    # pre-packed shared (replicated) weights
    wa_cp = np.ascontiguousarray(
        W_attn.T.reshape(16, 128, L).transpose(1, 0, 2).reshape(128, 16 * L)
    )
    wc_cp = np.ascontiguousarray(
        W_comb.T.reshape(16, 128, H).transpose(1, 0, 2).reshape(128, 16 * H)
    )
    b_comb_row = np.ascontiguousarray(b_comb.reshape(1, H))
    b_attn_r = b_attn.reshape(1, L)

    W_out_pad = np.zeros((VP, H), np.float32)
    W_out_pad[:V] = W_out
    WoT = np.ascontiguousarray(W_out_pad.T.astype(np.float16))  # [H, VP]
    b_out_pad = np.full(VP, PAD_BIAS, np.float32)
    b_out_pad[:V] = b_out
    bo_cp = np.ascontiguousarray(b_out_pad.reshape(128, NB))  # row p = vocab p*NB..

    h_nat = hidden.reshape(H)
    wihT = np.ascontiguousarray(w_ih.T)              # [H, 3H]
    whhT = np.ascontiguousarray(w_hh.T)

    emb_pad_last = np.zeros((VE, H), np.float32)
    emb_pad_last[: V - 7 * VE] = emb[7 * VE :]

    in_maps = []
    for c in range(M):
        s = slice(c * 128, (c + 1) * 128)
        wih_c = np.concatenate(
            [wihT[:, g * H + c * 128 : g * H + (c + 1) * 128] for g in range(3)],
            axis=1,
        )  # [H, 384]
        whh_c = np.concatenate(
            [whhT[:, g * H + c * 128 : g * H + (c + 1) * 128] for g in range(3)],
            axis=1,
        )
        wih_cp = np.ascontiguousarray(
            wih_c.reshape(NC_H, 128, 384).transpose(1, 0, 2).reshape(128, NC_H * 384)
        )
        whh_cp = np.ascontiguousarray(
            whh_c.reshape(NC_H, 128, 384).transpose(1, 0, 2).reshape(128, NC_H * 384)
        )
        bih_row = np.concatenate(
            [b_ih[g * H + c * 128 : g * H + (c + 1) * 128] for g in range(3)]
        ).reshape(1, 384)
        bhh_row = np.concatenate(
            [b_hh[g * H + c * 128 : g * H + (c + 1) * 128] for g in range(3)]
        ).reshape(1, 384)
        emb_shard = emb[c * VE : (c + 1) * VE] if c < 7 else emb_pad_last
        in_maps.append({
            "tok": np.full((2, 1), tok, np.int32),
            "base": np.full((2, 1), c * VE, np.int32),
            "emb_shard": np.ascontiguousarray(emb_shard),
            "h_nat": h_nat,
            "h_slice_row": np.ascontiguousarray(h_nat[s].reshape(1, 128)),
            "enc": enc,
            "wa_cp": wa_cp,
            "b_attn": b_attn_r,
            "wc_cp": wc_cp,
            "b_comb_row": b_comb_row,
            "wih_cp": wih_cp,
            "whh_cp": whh_cp,
            "bih_row": np.ascontiguousarray(bih_row),
            "bhh_row": np.ascontiguousarray(bhh_row),
            "wo_cp": np.ascontiguousarray(WoT[s]),
            "bo_cp": bo_cp,
        })
    return in_maps


def assemble(results):
    logp = results[0]["out_logp"][:V].reshape(1, V).astype(np.float32)
    h_new = np.concatenate(
        [results[c]["out_h"] for c in range(M)]
    ).reshape(1, 1, H).astype(np.float32)
    attn = results[0]["out_attn"].reshape(1, L).astype(np.float32)
    return logp, h_new, attn


def kernel(**inputs):
    nc = _get_nc()
    in_maps = prep_in_maps(**inputs)
    kw = {}
    if os.environ.get("KERNEL_TRACE"):
        kw = dict(trace=True, tmpdir=os.environ.get("KERNEL_TRACE_DIR") or None)
    res = run_bass_kernel_spmd(nc, in_maps, list(range(M)), **kw)
    if os.environ.get("KERNEL_TRACE"):
        print(f"HW exec time: {res.exec_time_ns} ns")
    return assemble(res.results)


# revision 35
# speedup vs baseline: 1.1043x; 1.0587x over previous
"""AttnDecoderRNN single-step kernel for 8 Trainium2 NeuronCores.

Parallelization (tensor-parallel over the vocab/hidden dims):
  - emb table row-sharded 8 ways; device-side masked indirect-DMA gather +
    AllReduce broadcasts the embedded row to all cores.
  - attention + combine replicated (W_attn, W_comb, encoder_outputs on every
    core) so each core computes the full attn/x vectors with no collective.
  - GRU gate weights (w_ih/w_hh) output-row-sharded: each core computes the
    gates and h_new only for its 128-dim slice of H, locally.
  - W_out column(H)-sharded: each core computes partial logits for the FULL
    (padded) vocab from its h_new slice; one AllReduce sums the partials,
    then every core runs the log_softmax epilogue identically.

All weights are host pre-packed into the chunk-partition layout [128, ...]
the matmuls consume (pure layout prep; all FLOPs run on device).
"""

import os
import sys

sys.path.insert(0, "/opt/trn_rl_repo")

import numpy as np

import concourse.bass as bass
import concourse.bacc as bacc
import concourse.mybir as mybir
import concourse.tile as tile
from concourse.bass_utils import run_bass_kernel_spmd
from concourse.masks import make_identity

F32 = mybir.dt.float32
F32R = mybir.dt.float32r
F16 = mybir.dt.float16
I32 = mybir.dt.int32
ALU = mybir.AluOpType
ACT = mybir.ActivationFunctionType
AX = mybir.AxisListType

H = 1024          # hidden size
L = 40            # encoder slots
V = 50257         # vocab
M = 8             # cores
VP = 53248        # padded vocab = 416 * 128
VE = VP // M      # emb rows per core      (6656)
NB = VP // 128    # vocab blocks           (416)
NC_H = H // 128   # h chunks               (8)
TW = 4096         # W_out tile width (cols per streamed tile)
NT = VP // TW     # streamed W_out tiles   (26)
PAD_BIAS = -30000.0

# ---------------------------------------------------------------- device ---


def build_program():
    nc = bacc.Bacc("TRN2", target_bir_lowering=False, num_devices=M)

    tok_t = nc.dram_tensor("tok", [2, 1], I32, kind="ExternalInput")
    base_t = nc.dram_tensor("base", [2, 1], I32, kind="ExternalInput")
    emb_t = nc.dram_tensor("emb_shard", [VE, H], F32, kind="ExternalInput")
    hnat_t = nc.dram_tensor("h_nat", [H], F32, kind="ExternalInput")
    hsl_t = nc.dram_tensor("h_slice_row", [1, 128], F32, kind="ExternalInput")
    enc_t = nc.dram_tensor("enc", [L, H], F32, kind="ExternalInput")
    wa_t = nc.dram_tensor("wa_cp", [128, 16 * L], F32, kind="ExternalInput")
    ba_t = nc.dram_tensor("b_attn", [1, L], F32, kind="ExternalInput")
    wc_t = nc.dram_tensor("wc_cp", [128, 16 * H], F32R, kind="ExternalInput")
    bc_t = nc.dram_tensor("b_comb_row", [1, H], F32, kind="ExternalInput")
    wih_t = nc.dram_tensor("wih_cp", [128, NC_H * 384], F32R, kind="ExternalInput")
    whh_t = nc.dram_tensor("whh_cp", [128, NC_H * 384], F32R, kind="ExternalInput")
    bih_t = nc.dram_tensor("bih_row", [1, 384], F32, kind="ExternalInput")
    bhh_t = nc.dram_tensor("bhh_row", [1, 384], F32, kind="ExternalInput")
    wo_t = nc.dram_tensor("wo_cp", [128, VP], F16, kind="ExternalInput")
    bo_t = nc.dram_tensor("bo_cp", [128, NB], F32, kind="ExternalInput")

    logp_t = nc.dram_tensor("out_logp", [VP], F32, kind="ExternalOutput")
    outh_t = nc.dram_tensor("out_h", [128], F32, kind="ExternalOutput")
    outa_t = nc.dram_tensor("out_attn", [L], F32, kind="ExternalOutput")

    with tile.TileContext(nc) as tc:
        with (
            tc.tile_pool(name="dram", bufs=1, space="DRAM") as dram,
            tc.tile_pool(name="w", bufs=1) as wpool,
            tc.tile_pool(name="s", bufs=1) as spool,
            tc.tile_pool(name="wo", bufs=7) as wo_pool,
        ):
            # ---- token -> masked embedding row gather, AllReduce first ----
            tok_sb = spool.tile([2, 1], I32)
            base_sb = spool.tile([2, 1], I32)
            nc.sync.dma_start(tok_sb[:], tok_t[:])
            nc.sync.dma_start(base_sb[:], base_t[:])
            d_sb = spool.tile([2, 1], I32)
            nc.vector.tensor_tensor(
                out=d_sb[:], in0=tok_sb[:], in1=base_sb[:], op=ALU.subtract
            )
            dcl = spool.tile([2, 1], I32)
            nc.vector.tensor_scalar(
                out=dcl[:], in0=d_sb[:], scalar1=0, scalar2=VE - 1,
                op0=ALU.max, op1=ALU.min,
            )
            d_f = spool.tile([2, 1], F32)
            nc.vector.tensor_copy(out=d_f[:], in_=d_sb[:])
            m1 = spool.tile([2, 1], F32)
            m2 = spool.tile([2, 1], F32)
            msk = spool.tile([2, 1], F32)
            nc.vector.tensor_scalar(
                out=m1[:], in0=d_f[:], scalar1=0.0, scalar2=None, op0=ALU.is_ge
            )
            nc.vector.tensor_scalar(
                out=m2[:], in0=d_f[:], scalar1=float(VE - 1), scalar2=None,
                op0=ALU.is_le,
            )
            nc.vector.tensor_tensor(out=msk[:], in0=m1[:], in1=m2[:], op=ALU.mult)
            gath = spool.tile([2, H], F32)
            nc.gpsimd.indirect_dma_start(
                out=gath[:],
                out_offset=None,
                in_=emb_t[:, :],
                in_offset=bass.IndirectOffsetOnAxis(ap=dcl[:, :1], axis=0),
            )
            erow = spool.tile([1, H], F32)
            nc.vector.tensor_scalar(
                out=erow[:], in0=gath[0:1, :], scalar1=msk[0:1, 0:1],
                scalar2=None, op0=ALU.mult,
            )
            em_in = dram.tile([1, H], F32)
            em_out = dram.tile([1, H], F32)
            nc.sync.dma_start(em_in[:], erow[:])
            nc.gpsimd.collective_compute(
                "AllReduce", ALU.add, replica_groups=[list(range(M))],
                ins=[em_in.opt()], outs=[em_out.opt()],
            )

            # ---- small weights into SBUF -------------------------------
            wa_sb = wpool.tile([128, 16 * L], F32)
            ba_sb = wpool.tile([1, L], F32)
            enc_sb = wpool.tile([L, H], F32)
            wc_sb = wpool.tile([128, 16 * H], F32R)
            bc_sb = wpool.tile([1, H], F32)
            wih_sb = wpool.tile([128, NC_H * 384], F32R)
            whh_sb = wpool.tile([128, NC_H * 384], F32R)
            bih_sb = wpool.tile([1, 384], F32)
            bhh_sb = wpool.tile([1, 384], F32)
            hsl_sb = wpool.tile([1, 128], F32)
            bo_sb = wpool.tile([128, NB], F32)
            ident = wpool.tile([128, 128], F32)
            ones1 = wpool.tile([1, 128], F32)
            nc.gpsimd.memset(ones1[:], 1.0)
            nc.sync.dma_start(wa_sb[:], wa_t[:])
            nc.sync.dma_start(ba_sb[:], ba_t[:])
            nc.sync.dma_start(enc_sb[:], enc_t[:])
            nc.scalar.dma_start(wc_sb[:], wc_t[:])
            nc.scalar.dma_start(bc_sb[:], bc_t[:])
            nc.sync.dma_start(wih_sb[:], wih_t[:])
            nc.scalar.dma_start(whh_sb[:], whh_t[:])
            nc.sync.dma_start(bih_sb[:], bih_t[:])
            nc.scalar.dma_start(bhh_sb[:], bhh_t[:])
            nc.sync.dma_start(hsl_sb[:], hsl_t[:])
            nc.sync.dma_start(bo_sb[:], bo_t[:])
            make_identity(nc, ident[:])

            hin = spool.tile([128, NC_H], F32)
            nc.sync.dma_start(hin[:], hnat_t[:].rearrange("(c p) -> p c", p=128))
            ein = spool.tile([128, NC_H], F32)
            nc.sync.dma_start(
                ein[:], em_out[:].rearrange("a (c p) -> p (a c)", p=128)
            )
            ein_r = spool.tile([128, NC_H], F32R)
            nc.vector.tensor_copy(out=ein_r[:], in_=ein[:])
            hin_r = spool.tile([128, NC_H], F32R)
            nc.vector.tensor_copy(out=hin_r[:], in_=hin[:])

            # ---- attention --------------------------------------------
            with tc.tile_pool(name="ps_a", bufs=1, space="PSUM") as ps_a:
                sc_ps = ps_a.tile([1, L], F32, space="PSUM")
                for c in range(16):
                    lhsT = ein[:, c : c + 1] if c < 8 else hin[:, c - 8 : c - 7]
                    nc.tensor.matmul(
                        sc_ps[0:1, 0:L], lhsT=lhsT,
                        rhs=wa_sb[:, c * L : (c + 1) * L],
                        start=(c == 0), stop=(c == 15),
                    )
                sc_sb = spool.tile([1, L], F32)
                nc.vector.tensor_tensor(
                    out=sc_sb[:], in0=sc_ps[:], in1=ba_sb[:], op=ALU.add
                )
                mx = spool.tile([1, 1], F32)
                nc.vector.tensor_reduce(
                    out=mx[:], in_=sc_sb[:], axis=AX.X, op=ALU.max
                )
                nmx = spool.tile([1, 1], F32)
                nc.vector.tensor_scalar(
                    out=nmx[:], in0=mx[:], scalar1=-1.0, scalar2=None, op0=ALU.mult
                )
                aw_e = spool.tile([1, L], F32)
                ssum = spool.tile([1, 1], F32)
                nc.scalar.activation(
                    out=aw_e[:], in_=sc_sb[:], func=ACT.Exp,
                    bias=nmx[0:1, 0:1], accum_out=ssum[0:1, 0:1],
                )
                rs = spool.tile([1, 1], F32)
                nc.vector.reciprocal(out=rs[:], in_=ssum[:])
                aw = spool.tile([1, L], F32)
                nc.vector.tensor_scalar(
                    out=aw[:], in0=aw_e[:], scalar1=rs[0:1, 0:1], scalar2=None,
                    op0=ALU.mult,
                )
                nc.sync.dma_start(
                    outa_t[:].rearrange("(a l) -> a l", a=1), aw[0:1, 0:L]
                )
                awT_ps = ps_a.tile([L, 1], F32, space="PSUM")
                nc.tensor.transpose(
                    out=awT_ps[0:L, 0:1], in_=aw[0:1, 0:L], identity=ident[0:1, 0:1]
                )
                awT = spool.tile([L, 1], F32)
                nc.vector.tensor_copy(out=awT[:], in_=awT_ps[:])
                aa_ps = ps_a.tile([128, NC_H], F32, space="PSUM")
                for c2 in range(NC_H):
                    nc.tensor.matmul(
                        aa_ps[:, c2 : c2 + 1],
                        lhsT=enc_sb[:, c2 * 128 : (c2 + 1) * 128],
                        rhs=awT[0:L, 0:1], start=True, stop=True,
                    )
                aa_sb = spool.tile([128, NC_H], F32R)
                nc.vector.tensor_copy(out=aa_sb[:], in_=aa_ps[:])

            # ---- combine + GRU: vector-stationary fp32r matmuls -------
            # x.T [1, H] = sum_kc cin_kc.T @ WcT slab (weights moving, N=512)
            with tc.tile_pool(name="ps_g", bufs=1, space="PSUM") as ps_g:
                xt_ps0 = ps_g.tile([1, 512], F32, space="PSUM")
                xt_ps1 = ps_g.tile([1, 512], F32, space="PSUM")
                for kc in range(16):
                    lhsT = (
                        ein_r[:, kc : kc + 1]
                        if kc < 8
                        else aa_sb[:, kc - 8 : kc - 7]
                    )
                    for half, xps in ((0, xt_ps0), (1, xt_ps1)):
                        nc.tensor.matmul(
                            xps[0:1, :],
                            lhsT=lhsT,
                            rhs=wc_sb[
                                :, kc * H + half * 512 : kc * H + half * 512 + 512
                            ],
                            start=(kc == 0), stop=(kc == 15),
                        )
                xt = spool.tile([1, H], F32)
                nc.vector.tensor_tensor(
                    out=xt[:, 0:512], in0=xt_ps0[0:1, :], in1=bc_sb[:, 0:512],
                    op=ALU.add,
                )
                nc.vector.tensor_tensor(
                    out=xt[:, 512:H], in0=xt_ps1[0:1, :], in1=bc_sb[:, 512:H],
                    op=ALU.add,
                )
                xr = spool.tile([1, H], F32)
                nc.scalar.activation(out=xr[:], in_=xt[:], func=ACT.Relu)
                # transpose x back to chunk-partition [128, 8] for lhsT use
                xc_ps = ps_g.tile([128, NC_H], F32, space="PSUM")
                for c in range(NC_H):
                    nc.tensor.transpose(
                        out=xc_ps[:, c : c + 1],
                        in_=xr[0:1, c * 128 : (c + 1) * 128],
                        identity=ident[0:1, 0:1],
                    )
                x_cp = spool.tile([128, NC_H], F32R)
                nc.vector.tensor_copy(out=x_cp[:], in_=xc_ps[:])

                gi_ps = ps_g.tile([1, 384], F32, space="PSUM")
                gh_ps = ps_g.tile([1, 384], F32, space="PSUM")
                for kc in range(NC_H):
                    nc.tensor.matmul(
                        gi_ps[0:1, :],
                        lhsT=x_cp[:, kc : kc + 1],
                        rhs=wih_sb[:, kc * 384 : (kc + 1) * 384],
                        start=(kc == 0), stop=(kc == NC_H - 1),
                    )
                for kc in range(NC_H):
                    nc.tensor.matmul(
                        gh_ps[0:1, :],
                        lhsT=hin_r[:, kc : kc + 1],
                        rhs=whh_sb[:, kc * 384 : (kc + 1) * 384],
                        start=(kc == 0), stop=(kc == NC_H - 1),
                    )
                gi = spool.tile([1, 384], F32)
                gh = spool.tile([1, 384], F32)
                nc.vector.tensor_tensor(
                    out=gi[:], in0=gi_ps[:], in1=bih_sb[:], op=ALU.add
                )
                nc.vector.tensor_tensor(
                    out=gh[:], in0=gh_ps[:], in1=bhh_sb[:], op=ALU.add
                )
            rz_in = spool.tile([1, 256], F32)
            nc.vector.tensor_tensor(
                out=rz_in[:], in0=gi[:, 0:256], in1=gh[:, 0:256], op=ALU.add
            )
            rz = spool.tile([1, 256], F32)
            nc.scalar.activation(out=rz[:], in_=rz_in[:], func=ACT.Sigmoid)
            rhn = spool.tile([1, 128], F32)
            nc.vector.tensor_tensor(
                out=rhn[:], in0=rz[:, 0:128], in1=gh[:, 256:384], op=ALU.mult
            )
            nin = spool.tile([1, 128], F32)
            nc.vector.tensor_tensor(
                out=nin[:], in0=gi[:, 256:384], in1=rhn[:], op=ALU.add
            )
            nn = spool.tile([1, 128], F32)
            nc.scalar.activation(out=nn[:], in_=nin[:], func=ACT.Tanh)
            hmn = spool.tile([1, 128], F32)
            nc.vector.tensor_tensor(
                out=hmn[:], in0=hsl_sb[:], in1=nn[:], op=ALU.subtract
            )
            zh = spool.tile([1, 128], F32)
            nc.vector.tensor_tensor(
                out=zh[:], in0=rz[:, 128:256], in1=hmn[:], op=ALU.mult
            )
            hnew = spool.tile([1, 128], F32)
            nc.vector.tensor_tensor(out=hnew[:], in0=nn[:], in1=zh[:], op=ALU.add)
            nc.sync.dma_start(outh_t[:].rearrange("(a p) -> a p", a=1), hnew[:])

            # ---- W_out partial logits: h stationary, W moving fp32r ---
            ar_in = dram.tile([VP], F32)
            ar_out = dram.tile([VP], F32)
            with tc.tile_pool(name="ps_h", bufs=1, space="PSUM") as ps_h:
                hc_ps = ps_h.tile([128, 1], F32, space="PSUM")
                nc.tensor.transpose(
                    out=hc_ps[:, 0:1], in_=hnew[0:1, :], identity=ident[0:1, 0:1]
                )
                hnew_cp = spool.tile([128, 1], F16)
                nc.vector.tensor_copy(out=hnew_cp[:], in_=hc_ps[:])
            with (
                tc.tile_pool(name="ps_lg", bufs=8, space="PSUM") as ps_lg,
                tc.tile_pool(name="lgstage", bufs=6) as lgstage,
            ):
                for t in range(NT):
                    wtile = wo_pool.tile([128, TW], F16)
                    # alternate the weight stream across both HWDGE engines
                    weng = nc.sync if t % 2 == 0 else nc.scalar
                    weng.dma_start(wtile[:], wo_t[:, t * TW : (t + 1) * TW])
                    for s in range(TW // 512):
                        lgt_ps = ps_lg.tile(
                            [1, 512], F32, space="PSUM", tag="lgt"
                        )
                        nc.tensor.matmul(
                            lgt_ps[0:1, :],
                            lhsT=hnew_cp[:, 0:1],
                            rhs=wtile[:, s * 512 : (s + 1) * 512],
                            start=True, stop=True,
                        )
                        lgt_sb = lgstage.tile([1, 512], F32, tag="lgs")
                        nc.vector.tensor_copy(out=lgt_sb[:], in_=lgt_ps[:])
                        off = t * TW + s * 512
                        nc.gpsimd.dma_start(
                            ar_in[off : off + 512].rearrange(
                                "(a n) -> a n", a=1
                            ),
                            lgt_sb[:],
                        )
            nc.gpsimd.collective_compute(
                "AllReduce", ALU.add, replica_groups=[list(range(M))],
                ins=[ar_in.opt()], outs=[ar_out.opt()],
            )

            # ---- log_softmax epilogue (identical on all cores) --------
            lgf = spool.tile([128, NB], F32)
            nc.sync.dma_start(
                lgf[:], ar_out[:].rearrange("(p b) -> p b", p=128)
            )
            lgb = spool.tile([128, NB], F32)
            nc.vector.tensor_tensor(out=lgb[:], in0=lgf[:], in1=bo_sb[:], op=ALU.add)
            rmx = spool.tile([128, 1], F32)
            nc.vector.tensor_reduce(out=rmx[:], in_=lgb[:], axis=AX.X, op=ALU.max)
            gmx = spool.tile([1, 1], F32)
            ngmx = spool.tile([128, 1], F32)
            ex = spool.tile([128, NB], F32)
            rsum = spool.tile([128, 1], F32)
            gsum = spool.tile([1, 1], F32)
            lz = spool.tile([1, 1], F32)
            logz = spool.tile([1, 1], F32)
            logz_sb = spool.tile([128, 1], F32)
            logp = spool.tile([128, NB], F32)
            with tc.tile_pool(name="ps_b", bufs=1, space="PSUM") as ps_b:
                rmxT_ps = ps_b.tile([1, 128], F32, space="PSUM")
                nc.tensor.transpose(
                    out=rmxT_ps[0:1, :], in_=rmx[:, 0:1], identity=ident[:, :]
                )
                nc.vector.tensor_reduce(
                    out=gmx[:], in_=rmxT_ps[0:1, :], axis=AX.X, op=ALU.max
                )
                gmxb_ps = ps_b.tile([128, 1], F32, space="PSUM")
                nc.tensor.matmul(
                    gmxb_ps[:, 0:1], lhsT=ones1[0:1, :], rhs=gmx[0:1, 0:1],
                    start=True, stop=True,
                )
                nc.vector.tensor_scalar(
                    out=ngmx[:], in0=gmxb_ps[:, 0:1],
                    scalar1=-1.0, scalar2=None, op0=ALU.mult,
                )
                nc.scalar.activation(
                    out=ex[:], in_=lgb[:], func=ACT.Exp,
                    bias=ngmx[:, 0:1], accum_out=rsum[:, 0:1],
                )
                rsumT_ps = ps_b.tile([1, 128], F32, space="PSUM")
                nc.tensor.transpose(
                    out=rsumT_ps[0:1, :], in_=rsum[:, 0:1], identity=ident[:, :]
                )
                nc.vector.tensor_reduce(
                    out=gsum[:], in_=rsumT_ps[0:1, :], axis=AX.X, op=ALU.add
                )
                nc.scalar.activation(out=lz[:], in_=gsum[:], func=ACT.Ln)
                nc.vector.tensor_tensor(
                    out=logz[:], in0=gmx[:], in1=lz[:], op=ALU.add
                )
                logzb_ps = ps_b.tile([128, 1], F32, space="PSUM")
                nc.tensor.matmul(
                    logzb_ps[:, 0:1], lhsT=ones1[0:1, :], rhs=logz[0:1, 0:1],
                    start=True, stop=True,
                )
                nc.vector.tensor_copy(out=logz_sb[:], in_=logzb_ps[:, 0:1])
            nc.vector.tensor_scalar(
                out=logp[:], in0=lgb[:],
                scalar1=logz_sb[:, 0:1],
                scalar2=None, op0=ALU.subtract,
            )
            nc.sync.dma_start(
                logp_t[:].rearrange("(p b) -> p b", p=128), logp[:]
            )

    nc.compile()
    return nc


# ------------------------------------------------------------------ host ---

_NC = None


def _get_nc():
    global _NC
    if _NC is None:
        _NC = build_program()
    return _NC


def prep_in_maps(input_tok, hidden, encoder_outputs, emb, W_attn, b_attn,
                 W_comb, b_comb, w_ih, w_hh, b_ih, b_hh, W_out, b_out):
    f = lambda a: np.ascontiguousarray(np.asarray(a, dtype=np.float32))
    emb = f(emb)
    W_attn, b_attn = f(W_attn), f(b_attn)
    W_comb, b_comb = f(W_comb), f(b_comb)
    w_ih, w_hh, b_ih, b_hh = f(w_ih), f(w_hh), f(b_ih), f(b_hh)
    W_out, b_out = f(W_out), f(b_out)
    hidden = f(hidden)
    enc = f(encoder_outputs)
    tok = int(np.asarray(input_tok).reshape(-1)[0])

    # pre-packed shared (replicated) weights
    wa_cp = np.ascontiguousarray(
        W_attn.T.reshape(16, 128, L).transpose(1, 0, 2).reshape(128, 16 * L)
    )
    wc_cp = np.ascontiguousarray(
        W_comb.T.reshape(16, 128, H).transpose(1, 0, 2).reshape(128, 16 * H)
    )
    b_comb_row = np.ascontiguousarray(b_comb.reshape(1, H))
    b_attn_r = b_attn.reshape(1, L)

    W_out_pad = np.zeros((VP, H), np.float32)
    W_out_pad[:V] = W_out
    WoT = np.ascontiguousarray(W_out_pad.T.astype(np.float16))  # [H, VP]
    b_out_pad = np.full(VP, PAD_BIAS, np.float32)
    b_out_pad[:V] = b_out
    bo_cp = np.ascontiguousarray(b_out_pad.reshape(128, NB))  # row p = vocab p*NB..

    h_nat = hidden.reshape(H)
    wihT = np.ascontiguousarray(w_ih.T)              # [H, 3H]
    whhT = np.ascontiguousarray(w_hh.T)

    emb_pad_last = np.zeros((VE, H), np.float32)
    emb_pad_last[: V - 7 * VE] = emb[7 * VE :]

    in_maps = []
    for c in range(M):
        s = slice(c * 128, (c + 1) * 128)
        wih_c = np.concatenate(
            [wihT[:, g * H + c * 128 : g * H + (c + 1) * 128] for g in range(3)],
            axis=1,
        )  # [H, 384]
        whh_c = np.concatenate(
            [whhT[:, g * H + c * 128 : g * H + (c + 1) * 128] for g in range(3)],
            axis=1,
        )
        wih_cp = np.ascontiguousarray(
            wih_c.reshape(NC_H, 128, 384).transpose(1, 0, 2).reshape(128, NC_H * 384)
        )
        whh_cp = np.ascontiguousarray(
            whh_c.reshape(NC_H, 128, 384).transpose(1, 0, 2).reshape(128, NC_H * 384)
        )
        bih_row = np.concatenate(
            [b_ih[g * H + c * 128 : g * H + (c + 1) * 128] for g in range(3)]
        ).reshape(1, 384)
        bhh_row = np.concatenate(
            [b_hh[g * H + c * 128 : g * H + (c + 1) * 128] for g in range(3)]
        ).reshape(1, 384)
        emb_shard = emb[c * VE : (c + 1) * VE] if c < 7 else emb_pad_last
        in_maps.append({
            "tok": np.full((2, 1), tok, np.int32),
            "base": np.full((2, 1), c * VE, np.int32),
            "emb_shard": np.ascontiguousarray(emb_shard),
            "h_nat": h_nat,
            "h_slice_row": np.ascontiguousarray(h_nat[s].reshape(1, 128)),
            "enc": enc,
            "wa_cp": wa_cp,
            "b_attn": b_attn_r,
            "wc_cp": wc_cp,
            "b_comb_row": b_comb_row,
            "wih_cp": wih_cp,
            "whh_cp": whh_cp,
            "bih_row": np.ascontiguousarray(bih_row),
            "bhh_row": np.ascontiguousarray(bhh_row),
            "wo_cp": np.ascontiguousarray(WoT[s]),
            "bo_cp": bo_cp,
        })
    return in_maps


def assemble(results):
    logp = results[0]["out_logp"][:V].reshape(1, V).astype(np.float32)
    h_new = np.concatenate(
        [results[c]["out_h"] for c in range(M)]
    ).reshape(1, 1, H).astype(np.float32)
    attn = results[0]["out_attn"].reshape(1, L).astype(np.float32)
    return logp, h_new, attn


def kernel(**inputs):
    nc = _get_nc()
    in_maps = prep_in_maps(**inputs)
    kw = {}
    if os.environ.get("KERNEL_TRACE"):
        kw = dict(trace=True, tmpdir=os.environ.get("KERNEL_TRACE_DIR") or None)
    res = run_bass_kernel_spmd(nc, in_maps, list(range(M)), **kw)
    if os.environ.get("KERNEL_TRACE"):
        print(f"HW exec time: {res.exec_time_ns} ns")
    return assemble(res.results)


# revision 36
# speedup vs baseline: 1.2301x; 1.1139x over previous
"""AttnDecoderRNN single-step kernel for 8 Trainium2 NeuronCores.

Parallelization (tensor-parallel over the vocab/hidden dims):
  - emb table row-sharded 8 ways; device-side masked indirect-DMA gather +
    AllReduce broadcasts the embedded row to all cores.
  - attention + combine replicated (W_attn, W_comb, encoder_outputs on every
    core) so each core computes the full attn/x vectors with no collective.
  - GRU gate weights (w_ih/w_hh) output-row-sharded: each core computes the
    gates and h_new only for its 128-dim slice of H, locally.
  - W_out column(H)-sharded: each core computes partial logits for the FULL
    (padded) vocab from its h_new slice; one AllReduce sums the partials,
    then every core runs the log_softmax epilogue identically.

All weights are host pre-packed into the chunk-partition layout [128, ...]
the matmuls consume (pure layout prep; all FLOPs run on device).
"""

import os
import sys

sys.path.insert(0, "/opt/trn_rl_repo")

import numpy as np

import concourse.bass as bass
import concourse.bacc as bacc
import concourse.mybir as mybir
import concourse.tile as tile
from concourse.bass_utils import run_bass_kernel_spmd
from concourse.masks import make_identity

F32 = mybir.dt.float32
F32R = mybir.dt.float32r
F16 = mybir.dt.float16
I32 = mybir.dt.int32
ALU = mybir.AluOpType
ACT = mybir.ActivationFunctionType
AX = mybir.AxisListType

H = 1024          # hidden size
L = 40            # encoder slots
V = 50257         # vocab
M = 8             # cores
VP = 53248        # padded vocab = 416 * 128
VE = VP // M      # emb rows per core      (6656)
NB = VP // 128    # vocab blocks           (416)
NC_H = H // 128   # h chunks               (8)
TW = 4096         # W_out tile width (cols per streamed tile)
NT = VP // TW     # streamed W_out tiles   (26)
PAD_BIAS = -30000.0

# ---------------------------------------------------------------- device ---


def build_program():
    nc = bacc.Bacc("TRN2", target_bir_lowering=False, num_devices=M)

    tok_t = nc.dram_tensor("tok", [2, 1], I32, kind="ExternalInput")
    base_t = nc.dram_tensor("base", [2, 1], I32, kind="ExternalInput")
    emb_t = nc.dram_tensor("emb_shard", [VE, H], F32, kind="ExternalInput")
    hnat_t = nc.dram_tensor("h_nat", [H], F32, kind="ExternalInput")
    hsl_t = nc.dram_tensor("h_slice_row", [1, 128], F32, kind="ExternalInput")
    enc_t = nc.dram_tensor("enc", [L, H], F32, kind="ExternalInput")
    wa_t = nc.dram_tensor("wa_cp", [128, 16 * L], F32, kind="ExternalInput")
    ba_t = nc.dram_tensor("b_attn", [1, L], F32, kind="ExternalInput")
    wc_t = nc.dram_tensor("wc_cp", [128, 16 * H], F16, kind="ExternalInput")
    bc_t = nc.dram_tensor("b_comb_row", [1, H], F32, kind="ExternalInput")
    wih_t = nc.dram_tensor("wih_cp", [128, NC_H * 384], F16, kind="ExternalInput")
    whh_t = nc.dram_tensor("whh_cp", [128, NC_H * 384], F16, kind="ExternalInput")
    bih_t = nc.dram_tensor("bih_row", [1, 384], F32, kind="ExternalInput")
    bhh_t = nc.dram_tensor("bhh_row", [1, 384], F32, kind="ExternalInput")
    wo_t = nc.dram_tensor("wo_cp", [128, VP], F16, kind="ExternalInput")
    bo_t = nc.dram_tensor("bo_cp", [128, NB], F32, kind="ExternalInput")

    logp_t = nc.dram_tensor("out_logp", [VP], F32, kind="ExternalOutput")
    outh_t = nc.dram_tensor("out_h", [128], F32, kind="ExternalOutput")
    outa_t = nc.dram_tensor("out_attn", [L], F32, kind="ExternalOutput")

    with tile.TileContext(nc) as tc:
        with (
            tc.tile_pool(name="dram", bufs=1, space="DRAM") as dram,
            tc.tile_pool(name="w", bufs=1) as wpool,
            tc.tile_pool(name="s", bufs=1) as spool,
            tc.tile_pool(name="wo", bufs=10) as wo_pool,
        ):
            # ---- token -> masked embedding row gather, AllReduce first ----
            tok_sb = spool.tile([2, 1], I32)
            base_sb = spool.tile([2, 1], I32)
            nc.sync.dma_start(tok_sb[:], tok_t[:])
            nc.sync.dma_start(base_sb[:], base_t[:])
            d_sb = spool.tile([2, 1], I32)
            nc.vector.tensor_tensor(
                out=d_sb[:], in0=tok_sb[:], in1=base_sb[:], op=ALU.subtract
            )
            dcl = spool.tile([2, 1], I32)
            nc.vector.tensor_scalar(
                out=dcl[:], in0=d_sb[:], scalar1=0, scalar2=VE - 1,
                op0=ALU.max, op1=ALU.min,
            )
            d_f = spool.tile([2, 1], F32)
            nc.vector.tensor_copy(out=d_f[:], in_=d_sb[:])
            m1 = spool.tile([2, 1], F32)
            m2 = spool.tile([2, 1], F32)
            msk = spool.tile([2, 1], F32)
            nc.vector.tensor_scalar(
                out=m1[:], in0=d_f[:], scalar1=0.0, scalar2=None, op0=ALU.is_ge
            )
            nc.vector.tensor_scalar(
                out=m2[:], in0=d_f[:], scalar1=float(VE - 1), scalar2=None,
                op0=ALU.is_le,
            )
            nc.vector.tensor_tensor(out=msk[:], in0=m1[:], in1=m2[:], op=ALU.mult)
            gath = spool.tile([2, H], F32)
            nc.gpsimd.indirect_dma_start(
                out=gath[:],
                out_offset=None,
                in_=emb_t[:, :],
                in_offset=bass.IndirectOffsetOnAxis(ap=dcl[:, :1], axis=0),
            )
            erow = spool.tile([1, H], F32)
            nc.vector.tensor_scalar(
                out=erow[:], in0=gath[0:1, :], scalar1=msk[0:1, 0:1],
                scalar2=None, op0=ALU.mult,
            )
            em_in = dram.tile([1, H], F32)
            em_out = dram.tile([1, H], F32)
            nc.sync.dma_start(em_in[:], erow[:])
            nc.gpsimd.collective_compute(
                "AllReduce", ALU.add, replica_groups=[list(range(M))],
                ins=[em_in.opt()], outs=[em_out.opt()],
            )

            # ---- small weights into SBUF -------------------------------
            wa_sb = wpool.tile([128, 16 * L], F32)
            ba_sb = wpool.tile([1, L], F32)
            enc_sb = wpool.tile([L, H], F32)
            wc_sb = wpool.tile([128, 16 * H], F16)
            bc_sb = wpool.tile([1, H], F32)
            wih_sb = wpool.tile([128, NC_H * 384], F16)
            whh_sb = wpool.tile([128, NC_H * 384], F16)
            bih_sb = wpool.tile([1, 384], F32)
            bhh_sb = wpool.tile([1, 384], F32)
            hsl_sb = wpool.tile([1, 128], F32)
            bo_sb = wpool.tile([128, NB], F32)
            ident = wpool.tile([128, 128], F32)
            ones1 = wpool.tile([1, 128], F32)
            nc.gpsimd.memset(ones1[:], 1.0)
            nc.sync.dma_start(wa_sb[:], wa_t[:])
            nc.sync.dma_start(ba_sb[:], ba_t[:])
            nc.sync.dma_start(enc_sb[:], enc_t[:])
            nc.scalar.dma_start(wc_sb[:], wc_t[:])
            nc.scalar.dma_start(bc_sb[:], bc_t[:])
            nc.sync.dma_start(wih_sb[:], wih_t[:])
            nc.scalar.dma_start(whh_sb[:], whh_t[:])
            nc.sync.dma_start(bih_sb[:], bih_t[:])
            nc.scalar.dma_start(bhh_sb[:], bhh_t[:])
            nc.sync.dma_start(hsl_sb[:], hsl_t[:])
            nc.sync.dma_start(bo_sb[:], bo_t[:])
            make_identity(nc, ident[:])

            hin = spool.tile([128, NC_H], F32)
            nc.sync.dma_start(hin[:], hnat_t[:].rearrange("(c p) -> p c", p=128))
            ein = spool.tile([128, NC_H], F32)
            nc.sync.dma_start(
                ein[:], em_out[:].rearrange("a (c p) -> p (a c)", p=128)
            )
            ein_r = spool.tile([128, NC_H], F16)
            nc.vector.tensor_copy(out=ein_r[:], in_=ein[:])
            hin_r = spool.tile([128, NC_H], F16)
            nc.vector.tensor_copy(out=hin_r[:], in_=hin[:])

            # ---- attention --------------------------------------------
            with tc.tile_pool(name="ps_a", bufs=1, space="PSUM") as ps_a:
                sc_ps = ps_a.tile([1, L], F32, space="PSUM")
                for c in range(16):
                    lhsT = ein[:, c : c + 1] if c < 8 else hin[:, c - 8 : c - 7]
                    nc.tensor.matmul(
                        sc_ps[0:1, 0:L], lhsT=lhsT,
                        rhs=wa_sb[:, c * L : (c + 1) * L],
                        start=(c == 0), stop=(c == 15),
                    )
                sc_sb = spool.tile([1, L], F32)
                nc.vector.tensor_tensor(
                    out=sc_sb[:], in0=sc_ps[:], in1=ba_sb[:], op=ALU.add
                )
                mx = spool.tile([1, 1], F32)
                nc.vector.tensor_reduce(
                    out=mx[:], in_=sc_sb[:], axis=AX.X, op=ALU.max
                )
                nmx = spool.tile([1, 1], F32)
                nc.vector.tensor_scalar(
                    out=nmx[:], in0=mx[:], scalar1=-1.0, scalar2=None, op0=ALU.mult
                )
                aw_e = spool.tile([1, L], F32)
                ssum = spool.tile([1, 1], F32)
                nc.scalar.activation(
                    out=aw_e[:], in_=sc_sb[:], func=ACT.Exp,
                    bias=nmx[0:1, 0:1], accum_out=ssum[0:1, 0:1],
                )
                rs = spool.tile([1, 1], F32)
                nc.vector.reciprocal(out=rs[:], in_=ssum[:])
                aw = spool.tile([1, L], F32)
                nc.vector.tensor_scalar(
                    out=aw[:], in0=aw_e[:], scalar1=rs[0:1, 0:1], scalar2=None,
                    op0=ALU.mult,
                )
                nc.sync.dma_start(
                    outa_t[:].rearrange("(a l) -> a l", a=1), aw[0:1, 0:L]
                )
                awT_ps = ps_a.tile([L, 1], F32, space="PSUM")
                nc.tensor.transpose(
                    out=awT_ps[0:L, 0:1], in_=aw[0:1, 0:L], identity=ident[0:1, 0:1]
                )
                awT = spool.tile([L, 1], F32)
                nc.vector.tensor_copy(out=awT[:], in_=awT_ps[:])
                aa_ps = ps_a.tile([128, NC_H], F32, space="PSUM")
                for c2 in range(NC_H):
                    nc.tensor.matmul(
                        aa_ps[:, c2 : c2 + 1],
                        lhsT=enc_sb[:, c2 * 128 : (c2 + 1) * 128],
                        rhs=awT[0:L, 0:1], start=True, stop=True,
                    )
                aa_sb = spool.tile([128, NC_H], F16)
                nc.vector.tensor_copy(out=aa_sb[:], in_=aa_ps[:])

            # ---- combine + GRU: vector-stationary fp32r matmuls -------
            # x.T [1, H] = sum_kc cin_kc.T @ WcT slab (weights moving, N=512)
            with tc.tile_pool(name="ps_g", bufs=1, space="PSUM") as ps_g:
                xt_ps0 = ps_g.tile([1, 512], F32, space="PSUM")
                xt_ps1 = ps_g.tile([1, 512], F32, space="PSUM")
                for kc in range(16):
                    lhsT = (
                        ein_r[:, kc : kc + 1]
                        if kc < 8
                        else aa_sb[:, kc - 8 : kc - 7]
                    )
                    for half, xps in ((0, xt_ps0), (1, xt_ps1)):
                        nc.tensor.matmul(
                            xps[0:1, :],
                            lhsT=lhsT,
                            rhs=wc_sb[
                                :, kc * H + half * 512 : kc * H + half * 512 + 512
                            ],
                            start=(kc == 0), stop=(kc == 15),
                        )
                xt = spool.tile([1, H], F32)
                nc.vector.tensor_tensor(
                    out=xt[:, 0:512], in0=xt_ps0[0:1, :], in1=bc_sb[:, 0:512],
                    op=ALU.add,
                )
                nc.vector.tensor_tensor(
                    out=xt[:, 512:H], in0=xt_ps1[0:1, :], in1=bc_sb[:, 512:H],
                    op=ALU.add,
                )
                xr = spool.tile([1, H], F32)
                nc.scalar.activation(out=xr[:], in_=xt[:], func=ACT.Relu)
                # transpose x back to chunk-partition [128, 8] for lhsT use
                xc_ps = ps_g.tile([128, NC_H], F32, space="PSUM")
                for c in range(NC_H):
                    nc.tensor.transpose(
                        out=xc_ps[:, c : c + 1],
                        in_=xr[0:1, c * 128 : (c + 1) * 128],
                        identity=ident[0:1, 0:1],
                    )
                x_cp = spool.tile([128, NC_H], F16)
                nc.vector.tensor_copy(out=x_cp[:], in_=xc_ps[:])

                gi_ps = ps_g.tile([1, 384], F32, space="PSUM")
                gh_ps = ps_g.tile([1, 384], F32, space="PSUM")
                for kc in range(NC_H):
                    nc.tensor.matmul(
                        gi_ps[0:1, :],
                        lhsT=x_cp[:, kc : kc + 1],
                        rhs=wih_sb[:, kc * 384 : (kc + 1) * 384],
                        start=(kc == 0), stop=(kc == NC_H - 1),
                    )
                for kc in range(NC_H):
                    nc.tensor.matmul(
                        gh_ps[0:1, :],
                        lhsT=hin_r[:, kc : kc + 1],
                        rhs=whh_sb[:, kc * 384 : (kc + 1) * 384],
                        start=(kc == 0), stop=(kc == NC_H - 1),
                    )
                gi = spool.tile([1, 384], F32)
                gh = spool.tile([1, 384], F32)
                nc.vector.tensor_tensor(
                    out=gi[:], in0=gi_ps[:], in1=bih_sb[:], op=ALU.add
                )
                nc.vector.tensor_tensor(
                    out=gh[:], in0=gh_ps[:], in1=bhh_sb[:], op=ALU.add
                )
            rz_in = spool.tile([1, 256], F32)
            nc.vector.tensor_tensor(
                out=rz_in[:], in0=gi[:, 0:256], in1=gh[:, 0:256], op=ALU.add
            )
            rz = spool.tile([1, 256], F32)
            nc.scalar.activation(out=rz[:], in_=rz_in[:], func=ACT.Sigmoid)
            rhn = spool.tile([1, 128], F32)
            nc.vector.tensor_tensor(
                out=rhn[:], in0=rz[:, 0:128], in1=gh[:, 256:384], op=ALU.mult
            )
            nin = spool.tile([1, 128], F32)
            nc.vector.tensor_tensor(
                out=nin[:], in0=gi[:, 256:384], in1=rhn[:], op=ALU.add
            )
            nn = spool.tile([1, 128], F32)
            nc.scalar.activation(out=nn[:], in_=nin[:], func=ACT.Tanh)
            hmn = spool.tile([1, 128], F32)
            nc.vector.tensor_tensor(
                out=hmn[:], in0=hsl_sb[:], in1=nn[:], op=ALU.subtract
            )
            zh = spool.tile([1, 128], F32)
            nc.vector.tensor_tensor(
                out=zh[:], in0=rz[:, 128:256], in1=hmn[:], op=ALU.mult
            )
            hnew = spool.tile([1, 128], F32)
            nc.vector.tensor_tensor(out=hnew[:], in0=nn[:], in1=zh[:], op=ALU.add)
            nc.sync.dma_start(outh_t[:].rearrange("(a p) -> a p", a=1), hnew[:])

            # ---- W_out partial logits: h stationary, W moving fp32r ---
            ar_in = dram.tile([VP], F32)
            ar_out = dram.tile([VP], F32)
            with tc.tile_pool(name="ps_h", bufs=1, space="PSUM") as ps_h:
                hc_ps = ps_h.tile([128, 1], F32, space="PSUM")
                nc.tensor.transpose(
                    out=hc_ps[:, 0:1], in_=hnew[0:1, :], identity=ident[0:1, 0:1]
                )
                hnew_cp = spool.tile([128, 1], F16)
                nc.vector.tensor_copy(out=hnew_cp[:], in_=hc_ps[:])
            with (
                tc.tile_pool(name="ps_lg", bufs=8, space="PSUM") as ps_lg,
                tc.tile_pool(name="lgstage", bufs=6) as lgstage,
            ):
                for t in range(NT):
                    wtile = wo_pool.tile([128, TW], F16)
                    # alternate the weight stream across both HWDGE engines
                    weng = nc.sync if t % 2 == 0 else nc.scalar
                    weng.dma_start(wtile[:], wo_t[:, t * TW : (t + 1) * TW])
                    for s in range(TW // 512):
                        lgt_ps = ps_lg.tile(
                            [1, 512], F32, space="PSUM", tag="lgt"
                        )
                        nc.tensor.matmul(
                            lgt_ps[0:1, :],
                            lhsT=hnew_cp[:, 0:1],
                            rhs=wtile[:, s * 512 : (s + 1) * 512],
                            start=True, stop=True,
                        )
                        lgt_sb = lgstage.tile([1, 512], F32, tag="lgs")
                        nc.vector.tensor_copy(out=lgt_sb[:], in_=lgt_ps[:])
                        off = t * TW + s * 512
                        nc.gpsimd.dma_start(
                            ar_in[off : off + 512].rearrange(
                                "(a n) -> a n", a=1
                            ),
                            lgt_sb[:],
                        )
            nc.gpsimd.collective_compute(
                "AllReduce", ALU.add, replica_groups=[list(range(M))],
                ins=[ar_in.opt()], outs=[ar_out.opt()],
            )

            # ---- log_softmax epilogue (identical on all cores) --------
            lgf = spool.tile([128, NB], F32)
            nc.sync.dma_start(
                lgf[:], ar_out[:].rearrange("(p b) -> p b", p=128)
            )
            lgb = spool.tile([128, NB], F32)
            nc.vector.tensor_tensor(out=lgb[:], in0=lgf[:], in1=bo_sb[:], op=ALU.add)
            rmx = spool.tile([128, 1], F32)
            nc.vector.tensor_reduce(out=rmx[:], in_=lgb[:], axis=AX.X, op=ALU.max)
            gmx = spool.tile([1, 1], F32)
            ngmx = spool.tile([128, 1], F32)
            ex = spool.tile([128, NB], F32)
            rsum = spool.tile([128, 1], F32)
            gsum = spool.tile([1, 1], F32)
            lz = spool.tile([1, 1], F32)
            logz = spool.tile([1, 1], F32)
            logz_sb = spool.tile([128, 1], F32)
            logp = spool.tile([128, NB], F32)
            with tc.tile_pool(name="ps_b", bufs=1, space="PSUM") as ps_b:
                rmxT_ps = ps_b.tile([1, 128], F32, space="PSUM")
                nc.tensor.transpose(
                    out=rmxT_ps[0:1, :], in_=rmx[:, 0:1], identity=ident[:, :]
                )
                nc.vector.tensor_reduce(
                    out=gmx[:], in_=rmxT_ps[0:1, :], axis=AX.X, op=ALU.max
                )
                gmxb_ps = ps_b.tile([128, 1], F32, space="PSUM")
                nc.tensor.matmul(
                    gmxb_ps[:, 0:1], lhsT=ones1[0:1, :], rhs=gmx[0:1, 0:1],
                    start=True, stop=True,
                )
                nc.vector.tensor_scalar(
                    out=ngmx[:], in0=gmxb_ps[:, 0:1],
                    scalar1=-1.0, scalar2=None, op0=ALU.mult,
                )
                nc.scalar.activation(
                    out=ex[:], in_=lgb[:], func=ACT.Exp,
                    bias=ngmx[:, 0:1], accum_out=rsum[:, 0:1],
                )
                rsumT_ps = ps_b.tile([1, 128], F32, space="PSUM")
                nc.tensor.transpose(
                    out=rsumT_ps[0:1, :], in_=rsum[:, 0:1], identity=ident[:, :]
                )
                nc.vector.tensor_reduce(
                    out=gsum[:], in_=rsumT_ps[0:1, :], axis=AX.X, op=ALU.add
                )
                nc.scalar.activation(out=lz[:], in_=gsum[:], func=ACT.Ln)
                nc.vector.tensor_tensor(
                    out=logz[:], in0=gmx[:], in1=lz[:], op=ALU.add
                )
                logzb_ps = ps_b.tile([128, 1], F32, space="PSUM")
                nc.tensor.matmul(
                    logzb_ps[:, 0:1], lhsT=ones1[0:1, :], rhs=logz[0:1, 0:1],
                    start=True, stop=True,
                )
                nc.vector.tensor_copy(out=logz_sb[:], in_=logzb_ps[:, 0:1])
            nc.vector.tensor_scalar(
                out=logp[:], in0=lgb[:],
                scalar1=logz_sb[:, 0:1],
                scalar2=None, op0=ALU.subtract,
            )
            nc.sync.dma_start(
                logp_t[:].rearrange("(p b) -> p b", p=128), logp[:]
            )

    nc.compile()
    return nc


# ------------------------------------------------------------------ host ---

_NC = None


def _get_nc():
    global _NC
    if _NC is None:
        _NC = build_program()
    return _NC


def prep_in_maps(input_tok, hidden, encoder_outputs, emb, W_attn, b_attn,
                 W_comb, b_comb, w_ih, w_hh, b_ih, b_hh, W_out, b_out):
    f = lambda a: np.ascontiguousarray(np.asarray(a, dtype=np.float32))
    emb = f(emb)
    W_attn, b_attn = f(W_attn), f(b_attn)
    W_comb, b_comb = f(W_comb), f(b_comb)
    w_ih, w_hh, b_ih, b_hh = f(w_ih), f(w_hh), f(b_ih), f(b_hh)
    W_out, b_out = f(W_out), f(b_out)
    hidden = f(hidden)
    enc = f(encoder_outputs)
    tok = int(np.asarray(input_tok).reshape(-1)[0])

    # pre-packed shared (replicated) weights
    wa_cp = np.ascontiguousarray(
        W_attn.T.reshape(16, 128, L).transpose(1, 0, 2).reshape(128, 16 * L)
    )
    wc_cp = np.ascontiguousarray(
        W_comb.T.reshape(16, 128, H).transpose(1, 0, 2).reshape(128, 16 * H)
    ).astype(np.float16)
    b_comb_row = np.ascontiguousarray(b_comb.reshape(1, H))
    b_attn_r = b_attn.reshape(1, L)

    W_out_pad = np.zeros((VP, H), np.float32)
    W_out_pad[:V] = W_out
    WoT = np.ascontiguousarray(W_out_pad.T.astype(np.float16))  # [H, VP]
    b_out_pad = np.full(VP, PAD_BIAS, np.float32)
    b_out_pad[:V] = b_out
    bo_cp = np.ascontiguousarray(b_out_pad.reshape(128, NB))  # row p = vocab p*NB..

    h_nat = hidden.reshape(H)
    wihT = np.ascontiguousarray(w_ih.T)              # [H, 3H]
    whhT = np.ascontiguousarray(w_hh.T)

    emb_pad_last = np.zeros((VE, H), np.float32)
    emb_pad_last[: V - 7 * VE] = emb[7 * VE :]

    in_maps = []
    for c in range(M):
        s = slice(c * 128, (c + 1) * 128)
        wih_c = np.concatenate(
            [wihT[:, g * H + c * 128 : g * H + (c + 1) * 128] for g in range(3)],
            axis=1,
        )  # [H, 384]
        whh_c = np.concatenate(
            [whhT[:, g * H + c * 128 : g * H + (c + 1) * 128] for g in range(3)],
            axis=1,
        )
        wih_cp = np.ascontiguousarray(
            wih_c.reshape(NC_H, 128, 384).transpose(1, 0, 2).reshape(128, NC_H * 384)
        ).astype(np.float16)
        whh_cp = np.ascontiguousarray(
            whh_c.reshape(NC_H, 128, 384).transpose(1, 0, 2).reshape(128, NC_H * 384)
        ).astype(np.float16)
        bih_row = np.concatenate(
            [b_ih[g * H + c * 128 : g * H + (c + 1) * 128] for g in range(3)]
        ).reshape(1, 384)
        bhh_row = np.concatenate(
            [b_hh[g * H + c * 128 : g * H + (c + 1) * 128] for g in range(3)]
        ).reshape(1, 384)
        emb_shard = emb[c * VE : (c + 1) * VE] if c < 7 else emb_pad_last
        in_maps.append({
            "tok": np.full((2, 1), tok, np.int32),
            "base": np.full((2, 1), c * VE, np.int32),
            "emb_shard": np.ascontiguousarray(emb_shard),
            "h_nat": h_nat,
            "h_slice_row": np.ascontiguousarray(h_nat[s].reshape(1, 128)),
            "enc": enc,
            "wa_cp": wa_cp,
            "b_attn": b_attn_r,
            "wc_cp": wc_cp,
            "b_comb_row": b_comb_row,
            "wih_cp": wih_cp,
            "whh_cp": whh_cp,
            "bih_row": np.ascontiguousarray(bih_row),
            "bhh_row": np.ascontiguousarray(bhh_row),
            "wo_cp": np.ascontiguousarray(WoT[s]),
            "bo_cp": bo_cp,
        })
    return in_maps


def assemble(results):
    logp = results[0]["out_logp"][:V].reshape(1, V).astype(np.float32)
    h_new = np.concatenate(
        [results[c]["out_h"] for c in range(M)]
    ).reshape(1, 1, H).astype(np.float32)
    attn = results[0]["out_attn"].reshape(1, L).astype(np.float32)
    return logp, h_new, attn


def kernel(**inputs):
    nc = _get_nc()
    in_maps = prep_in_maps(**inputs)
    kw = {}
    if os.environ.get("KERNEL_TRACE"):
        kw = dict(trace=True, tmpdir=os.environ.get("KERNEL_TRACE_DIR") or None)
    res = run_bass_kernel_spmd(nc, in_maps, list(range(M)), **kw)
    if os.environ.get("KERNEL_TRACE"):
        print(f"HW exec time: {res.exec_time_ns} ns")
    return assemble(res.results)
